# revision 7
# baseline (speedup 1.0000x reference)
"""Trainium2 Bass kernel v3 for CRFHead (dense-Gaussian mean-field CRF).

v2 materialized E = exp(-0.5 d2) once per core in f16 (34/72 j-blocks
SBUF-resident, 38 streamed via DRAM) and ran 11 TensorE matvec passes with
PSUM accumulating across 72 j-block matmuls.  v3 keeps that skeleton and
removes the scheduling stalls found in the v2 trace:

- ~11us inter-pass bubbles: the next pass's first matmul chained through
  [group-4 collective -> qc DMA -> counting-sem -> group-0 weight split].
  v3 emits the qc DMAs with a 2-chunk lag on the Scalar queue and the
  f16 weight splits at the END of the producing pass, so pass k+1's
  weights are ready before pass k's last matmul retires.
- PSUM WAW serialization at pass boundaries: Rps is one [34, ROWS] tile;
  odd passes accumulate into rows 0:M, even passes into rows 32:32+M, so
  the first matmuls of pass k+1 don't wait for pass k's PSUM readers.
- ACT table ping-pong (Ln/Exp per chunk in pass 1's tail, 1.28us per
  reload on the Scalar queue): the per-i 1/sqrt(deg) coefficients now
  come from the exact AllReduced deg via a host-provided shard mask
  (masked reduce over the SHARDS axis) instead of riding pass 1 as a
  ones-weight column; pass 1 shrinks to M=4 and uses only Exp.

Sharding: 8 cores = 2 images x 4-way split of the 9216 output pixels.
"""

import numpy as np

B, C, H, W = 2, 3, 96, 96
N = H * W                 # 9216 pixels
N_CORES = 8
SHARDS = 4                # cores per image
ROWS = N // SHARDS        # 2304 local output rows per core
NT = ROWS // 128          # 18 local 128-row tiles
JB = N // 128             # 72 global j-blocks
NRES = 34                 # j-blocks resident in SBUF (slots NSTREAM..71)
NSTREAM = JB - NRES       # j-blocks streamed from DRAM (slots 0..NSTREAM-1)
KDIM = 12
REFINE_ITERS = 10
RG = [[0, 1, 2, 3], [4, 5, 6, 7]]
ICHUNKS = [(0, 512), (512, 512), (1024, 512), (1536, 512), (2048, 256)]
BCHUNKS = [(0, 1152), (1152, 1152)]
BSUB = [(0, 512), (512, 512), (1024, 128)]
RSTAG = 16                # resident slots reserved for the chunk-major tail
STAG_PICK = (0, 4, 5, 5, 2)
NT4 = 4                   # max t-tiles per i-chunk

F16 = np.float16


def _f16(x):
    return np.asarray(x, dtype=F16).astype(np.float32)


def _split3_f16(w):
    w = np.asarray(w, np.float32)
    w1 = np.asarray(w, F16)
    d1 = w - w1.astype(np.float32)
    w2 = np.asarray(d1, F16)
    w3 = np.asarray(d1 - w2.astype(np.float32), F16)
    return w1, w2, w3


def _host_prep(imgs, masks):
    """Mirror the reference's quantization exactly in numpy fp32."""
    imgs = np.asarray(imgs, np.float32)
    masks = np.asarray(masks, np.float32)
    MEAN = np.array([0.485, 0.456, 0.406], np.float32)[None, :, None, None]
    STD = np.array([0.229, 0.224, 0.225], np.float32)[None, :, None, None]
    x = (imgs * STD + MEAN).transpose(0, 2, 3, 1) * np.float32(255.0)
    x = np.floor(np.clip(x, 0.0, 255.0))
    m = np.floor(np.clip(masks * np.float32(255.0) / np.float32(0.7), 0.0, 255.0))
    return x, m


def _image_data(img_q, mask_q):
    """Per-image full-N host arrays (global row order = row-major pixels)."""
    U = mask_q / (mask_q.max() + 1e-8)
    U = np.clip(U, 1e-6, 1.0 - 1e-6).reshape(N).astype(np.float32)
    logitU = np.log(U / (np.float32(1.0) - U)).astype(np.float32)

    ys, xs = np.meshgrid(np.arange(H, dtype=np.float32),
                         np.arange(W, dtype=np.float32), indexing="ij")
    xv = xs.reshape(N)
    yv = ys.reshape(N)
    c = img_q.reshape(N, 3).astype(np.float32)

    ax = (xv / np.float32(12.0)).astype(np.float32)
    ay = (yv / np.float32(12.0)).astype(np.float32)
    axh = _f16(ax); axl = _f16(ax - axh)
    ayh = _f16(ay); ayl = _f16(ay - ayh)
    r, g, b = _f16(c[:, 0]), _f16(c[:, 1]), _f16(c[:, 2])
    twos = np.full(N, 2.0, np.float32)

    sqxy = xv * xv + yv * yv
    sqrgb = (c * c).sum(axis=1)
    bias = (-sqrgb / np.float32(50.0) - sqxy / np.float32(7200.0)).astype(np.float32)
    whalf = (np.float32(12.5) * bias).astype(np.float32)   # w_i/2; ones-row = 2

    # stationary j-side rows x moving i-side rows -> PSUM holds
    # 25*(f_i . f_j) + w_i ; ACT applies scale 1/25 and per-partition bias_j
    j_rows = np.stack([r, g, b, axh, axh, axl, ayh, ayh, ayl,
                       twos, twos, twos]).astype(F16)           # [12, N]
    i_rows = np.stack([r, g, b, axh, axl, axh, ayh, ayl, ayh,
                       *_split3_f16(whalf)]).astype(F16)        # [12, N]
    return dict(U=U, logitU=logitU, bias=bias,
                j_rows=j_rows, i_rows=i_rows)


def _pb_index():
    """Global row index for [p, jb] layouts: j = (jb//NT)*ROWS + (jb%NT)*128 + p."""
    p = np.arange(128)[:, None]
    jb = np.arange(JB)[None, :]
    return (jb // NT) * ROWS + (jb % NT) * 128 + p      # [128, 72]


def _core_inputs(data, g):
    gidx = _pb_index()
    jlhsT = data["j_rows"][:, gidx.T.reshape(-1)]                  # [12, 72*128]
    isl = slice(g * ROWS, (g + 1) * ROWS)
    irhs = data["i_rows"][:, isl]                                  # [12, 2304]
    biasJ = np.ascontiguousarray(data["bias"][gidx], np.float32)   # [128, 72]
    Ufull = np.ascontiguousarray(data["U"][gidx], np.float32)      # [128, 72]
    lidx = gidx[:, g * NT:(g + 1) * NT]
    logitU = np.ascontiguousarray(data["logitU"][lidx], np.float32)  # [128, 18]
    ident = np.eye(8, dtype=np.float32)
    # one-hot shard mask, [128, SHARDS, NT] flattened as [128, JB]
    maskexp = np.zeros((128, SHARDS, NT), np.float32)
    maskexp[:, g, :] = 1.0
    return {
        "jlhsT": np.ascontiguousarray(jlhsT),
        "irhs": np.ascontiguousarray(irhs),
        "biasJ": biasJ,
        "Ufull": Ufull,
        "logitUl": logitU,
        "ident": ident,
        "maskexp": np.ascontiguousarray(maskexp),
    }


def make_in_maps(imgs, masks):
    x, m = _host_prep(imgs, masks)
    per_image = [_image_data(x[b], m[b]) for b in range(B)]
    in_maps = []
    for k in range(N_CORES):
        b, g = divmod(k, SHARDS)
        in_maps.append(_core_inputs(per_image[b], g))
    return in_maps


def assemble(results):
    out = np.empty((B, N), np.float32)
    p = np.arange(128)[:, None]
    t = np.arange(NT)[None, :]
    lidx = (t * 128 + p).reshape(-1)
    for k in range(N_CORES):
        b, g = divmod(k, SHARDS)
        flat = np.empty(ROWS, np.float32)
        flat[lidx] = np.asarray(results[k]["q_out"], np.float32).reshape(-1)
        out[b, g * ROWS:(g + 1) * ROWS] = flat
    return out.reshape(B, H, W)


def build_program():
    import concourse.bacc as bacc
    import concourse.mybir as mybir
    from concourse.tile import TileContext

    f32 = mybir.dt.float32
    f16 = mybir.dt.float16
    AF = mybir.ActivationFunctionType
    AX = mybir.AxisListType
    ALU = mybir.AluOpType

    nc = bacc.Bacc(num_devices=N_CORES)

    jlhsT_in = nc.dram_tensor("jlhsT", [KDIM, N], f16, kind="ExternalInput")
    irhs_in = nc.dram_tensor("irhs", [KDIM, ROWS], f16, kind="ExternalInput")
    biasJ_in = nc.dram_tensor("biasJ", [128, JB], f32, kind="ExternalInput")
    Ufull_in = nc.dram_tensor("Ufull", [128, JB], f32, kind="ExternalInput")
    logitU_in = nc.dram_tensor("logitUl", [128, NT], f32, kind="ExternalInput")
    ident_in = nc.dram_tensor("ident", [8, 8], f32, kind="ExternalInput")
    maskexp_in = nc.dram_tensor("maskexp", [128, SHARDS, NT], f32,
                                kind="ExternalInput")
    q_out_d = nc.dram_tensor("q_out", [128, NT], f32, kind="ExternalOutput")

    SCALE = float(np.float32(1.0) / np.float32(25.0))

    with TileContext(nc) as tc:
        with (
            tc.tile_pool(name="const", bufs=1) as cpool,
            tc.tile_pool(name="vec", bufs=2) as vp,
            tc.tile_pool(name="ring", bufs=2) as rr,
            tc.tile_pool(name="bounce", bufs=4) as bpl,
            tc.tile_pool(name="sdram", bufs=1, space="DRAM") as dpc,
            tc.tile_pool(name="dramit", bufs=2, space="DRAM") as dp,
            tc.tile_pool(name="gdram", bufs=5, space="DRAM") as gp,
        ):
            # ---- persistent SBUF ----
            jlhsT_sb = cpool.tile([KDIM, N], f16, tag="jlhsT")
            irhs_sb = cpool.tile([KDIM, ROWS], f16, tag="irhs")
            biasJ_sb = cpool.tile([128, JB], f32, tag="biasJ")
            Ufull_sb = cpool.tile([128, JB], f32, tag="Ufull")
            logitU_sb = cpool.tile([128, NT], f32, tag="logitU")
            ident_sb = cpool.tile([8, 8], f32, tag="ident")
            maskexp_sb = cpool.tile([128, SHARDS, NT], f32, tag="maskexp")
            S16 = cpool.tile([128, NRES * ROWS], f16, tag="S16")
            pacc = cpool.tile([128, JB, 2], f32, tag="pacc")
            deg_sb = cpool.tile([128, JB], f32, tag="deg")
            rsqd = cpool.tile([128, SHARDS, NT], f32, tag="rsqd")
            qw4 = cpool.tile([128, JB, 4], f16, tag="qw4")
            rsqd_l = cpool.tile([128, NT], f32, tag="rsqdl")
            Acoef = cpool.tile([128, NT], f32, tag="Acoef")
            Ccoef = cpool.tile([128, NT], f32, tag="Ccoef")

            S_dram = dpc.tile([NSTREAM, 128, ROWS], f16, tag="Sdram")

            nc.sync.dma_start(out=jlhsT_sb[:, :], in_=jlhsT_in[:, :])
            nc.sync.dma_start(out=irhs_sb[:, :], in_=irhs_in[:, :])
            nc.sync.dma_start(out=biasJ_sb[:, :], in_=biasJ_in[:, :])
            nc.sync.dma_start(out=Ufull_sb[:, :], in_=Ufull_in[:, :])
            nc.sync.dma_start(out=logitU_sb[:, :], in_=logitU_in[:, :])
            nc.sync.dma_start(out=ident_sb[:, :], in_=ident_in[:, :])
            nc.sync.dma_start(out=maskexp_sb[:, :, :], in_=maskexp_in[:, :, :])

            # warm up the collective path (first CC op pays ~30us of ring
            # init) while the build matmuls run
            ccw = dp.tile([128, 1], f32, tag="ccw")
            ccwo = dp.tile([SHARDS, 128, 1], f32, tag="ccwo")
            nc.sync.dma_start(out=ccw[:, :], in_=biasJ_sb[:, 0:1])
            nc.gpsimd.collective_compute(
                "AllGather", ALU.bypass, replica_groups=RG,
                ins=[ccw[:].opt()], outs=[ccwo[:].opt()],
            )

            # ================= build pass =================
            with tc.tile_pool(name="bpsum", bufs=2, space="PSUM") as bpp:
                for jb in range(JB):
                    resident = jb >= NSTREAM
                    lhs = jlhsT_sb[:, jb * 128:(jb + 1) * 128]
                    bt = None
                    if not resident:
                        bt = bpl.tile([128, ROWS], f16, tag="bounce")
                    for ci, (c0, cw) in enumerate(BCHUNKS):
                        ps = bpp.tile([128, 1152], f32, tag="bps")
                        for s0, sw in BSUB:
                            nc.tensor.matmul(
                                ps[:, s0:s0 + sw], lhs,
                                irhs_sb[:, c0 + s0:c0 + s0 + sw],
                                start=True, stop=True,
                            )
                        if resident:
                            base = (jb - NSTREAM) * ROWS + c0
                            dst = S16[:, base:base + cw]
                        else:
                            dst = bt[:, c0:c0 + cw]
                        nc.scalar.activation(
                            dst, ps[:, :cw], AF.Exp,
                            bias=biasJ_sb[:, jb:jb + 1], scale=SCALE,
                            accum_out=pacc[:, jb, ci:ci + 1],
                        )
                    if not resident:
                        nc.sync.dma_start(out=S_dram[jb, :, :], in_=bt[:, :])

            # ---- deg: local partials + 4-core AllReduce ----
            pdeg = vp.tile([128, JB], f32, tag="pdeg")
            nc.vector.tensor_reduce(pdeg[:, :], pacc[:, :, :], AX.X, ALU.add)
            pd_d = dp.tile([128, JB], f32, tag="pd")
            pd_o = dp.tile([128, JB], f32, tag="pdo")
            nc.sync.dma_start(out=pd_d[:, :], in_=pdeg[:, :])
            nc.gpsimd.collective_compute(
                "AllReduce", ALU.add, replica_groups=RG,
                ins=[pd_d[:].opt()], outs=[pd_o[:].opt()],
            )
            nc.sync.dma_start(out=deg_sb[:, :], in_=pd_o[:, :])

            ln_deg = vp.tile([128, JB], f32, tag="lndeg")
            nc.scalar.activation(ln_deg[:, :], deg_sb[:, :], AF.Ln)
            nc.scalar.activation(rsqd[:, :, :], ln_deg[:, :], AF.Exp, scale=-0.5)

            # per-i coefficients from the exact deg: select this core's
            # shard columns via the host-provided one-hot mask
            masked = vp.tile([128, SHARDS, NT], f32, tag="maskd")
            nc.vector.tensor_mul(masked[:, :, :], rsqd[:, :, :],
                                 maskexp_sb[:, :, :])
            nc.vector.tensor_reduce(rsqd_l[:, :],
                                    masked[:, :, :].transpose([0, 2, 1]),
                                    AX.X, ALU.add)
            nc.vector.tensor_scalar_mul(Acoef[:, :], rsqd_l[:, :], 10.0)

            def split_hi_lo(src_f32, dst_hi, dst_lo):
                """dst_hi = f16(src); dst_lo = f16(src - dst_hi); dsts strided."""
                nc.vector.tensor_copy(dst_hi, src_f32)
                lo = vp.tile([128, JB], f32, tag="splo")
                nc.vector.tensor_sub(lo[:, :], src_f32, dst_hi)
                nc.vector.tensor_copy(dst_lo, lo[:, :])

            # pass-1 weights: [rsqd_hi, rsqd_lo, (rsqd*U)_hi, (rsqd*U)_lo]
            wq0 = vp.tile([128, JB], f32, tag="wq0")
            nc.vector.tensor_mul(wq0[:, :], rsqd[:, :, :], Ufull_sb[:, :])
            split_hi_lo(rsqd[:, :, :], qw4[:, :, 0], qw4[:, :, 1])
            split_hi_lo(wq0[:, :], qw4[:, :, 2], qw4[:, :, 3])

            # Slot schedule: main region ordered by gather-group (t//4);
            # within a group, streamed/resident slots merge proportionally so
            # the stream DMA hides under resident compute.  The last RSTAG
            # resident slots form the chunk-major stagger tail.
            def slot_schedule():
                res = list(range(NSTREAM, JB))
                stag = []
                for tg in range(5):
                    cand = [jb for jb in res
                            if tg * 4 <= (jb % NT) < min(tg * 4 + 4, NT)]
                    cand.sort(key=lambda jb: -(jb % NT))
                    stag.extend(cand[:STAG_PICK[tg]])
                assert len(stag) == RSTAG
                sset = set(stag)
                main = []
                for t0, tw in ((0, 4), (4, 4), (8, 4), (12, 4), (16, 2)):
                    grp = [jb for jb in range(JB)
                           if t0 <= (jb % NT) < t0 + tw and jb not in sset]
                    S = [jb for jb in grp if jb < NSTREAM]
                    R = [jb for jb in grp if jb >= NSTREAM]
                    ns, nr = len(S), len(R)
                    i = j = 0
                    while i < ns or j < nr:
                        if i < ns and (j >= nr or i * (nr + 1) <= j * (ns + 1)):
                            main.append(S[i])
                            i += 1
                        elif j < nr:
                            main.append(R[j])
                            j += 1
                return main, stag

            # ================= matvec passes =================
            with (
                tc.tile_pool(name="rpsum", bufs=1, space="PSUM") as rpp,
                tc.tile_pool(name="tpsum", bufs=2, space="PSUM") as tpp,
                tc.tile_pool(name="qwp", bufs=2) as qwp,
            ):
                # one PSUM tile; odd passes use rows 0:M, even rows 32:32+M
                Rps_all = rpp.tile([34, ROWS], f32, tag="rps", name="RpsAll")

                def matvec_pass(M, wsel, finish_chunk, prow):
                    """R[0:M] accumulated over all 72 j-blocks into PSUM rows
                    [prow, prow+M); transposed result lands in Tsb
                    [128, NT, :M]; finish_chunk(Tsb, nt0, ntw) runs staggered
                    per chunk and may return a deferred emitter, which is
                    emitted with a 2-chunk lag (remaining ones at the end)."""
                    Rps = Rps_all[prow:prow + M, :]
                    Tps = tpp.tile([128, NT, 4], f32, tag="tps", name="Tps")
                    Tsb = vp.tile([128, NT, 4], f32, tag="tsb", name="Tsb")
                    main, stag = slot_schedule()
                    first = True
                    for jb in main:
                        if jb < NSTREAM:
                            st = bpl.tile([128, ROWS], f16, tag="bounce")
                            nc.sync.dma_start(out=st[:, :], in_=S_dram[jb, :, :])
                            src, base0 = st, 0
                        else:
                            src, base0 = S16, (jb - NSTREAM) * ROWS
                        for c0, cw in ICHUNKS:
                            nc.tensor.matmul(
                                Rps[0:M, c0:c0 + cw], wsel(jb, M),
                                src[:, base0 + c0:base0 + c0 + cw],
                                start=first, stop=False,
                            )
                        first = False
                    deferred = []
                    for ci, (c0, cw) in enumerate(ICHUNKS):
                        for si, jb in enumerate(stag):
                            base = (jb - NSTREAM) * ROWS + c0
                            nc.tensor.matmul(
                                Rps[0:M, c0:c0 + cw], wsel(jb, M),
                                S16[:, base:base + cw],
                                start=False, stop=(si == RSTAG - 1),
                            )
                        rg = rr.tile([4, 512], f32, tag="rring")
                        nc.vector.tensor_copy(rg[0:M, 0:cw], Rps[0:M, c0:c0 + cw])
                        nt0, ntw = c0 // 128, cw // 128
                        for tt in range(ntw):
                            nc.tensor.transpose(
                                Tps[:, nt0 + tt, 0:M],
                                rg[0:M, tt * 128:(tt + 1) * 128],
                                ident_sb[0:M, 0:M],
                            )
                        nc.vector.tensor_copy(Tsb[:, nt0:nt0 + ntw, 0:M],
                                              Tps[:, nt0:nt0 + ntw, 0:M])
                        if finish_chunk is not None:
                            d = finish_chunk(Tsb, nt0, ntw)
                            if d is not None:
                                deferred.append(d)
                        if ci >= 2 and len(deferred) > ci - 2:
                            deferred[ci - 2]()
                            deferred[ci - 2] = lambda: None
                    for d in deferred:
                        d()
                    return Tsb

                def chunk_z(Tsb, nt0, ntw, qt, m0):
                    """z = Ccoef + Acoef * (T[m0]+T[m0+1]);
                    q slice <- 1/(1+exp(-z)) (keeps ACT on the Exp set)."""
                    Rr = vp.tile([128, NT], f32, tag="Rrc")
                    nc.vector.tensor_add(Rr[:, 0:ntw], Tsb[:, nt0:nt0 + ntw, m0],
                                         Tsb[:, nt0:nt0 + ntw, m0 + 1])
                    t2 = vp.tile([128, NT], f32, tag="t2c")
                    nc.vector.tensor_mul(t2[:, 0:ntw], Acoef[:, nt0:nt0 + ntw],
                                         Rr[:, 0:ntw])
                    z = vp.tile([128, NT], f32, tag="zc")
                    nc.vector.tensor_add(z[:, 0:ntw], Ccoef[:, nt0:nt0 + ntw],
                                         t2[:, 0:ntw])
                    ez = vp.tile([128, NT], f32, tag="ezc")
                    nc.scalar.activation(ez[:, 0:ntw], z[:, 0:ntw],
                                         AF.Exp, scale=-1.0)
                    e1 = vp.tile([128, NT], f32, tag="e1c")
                    nc.vector.tensor_scalar_add(e1[:, 0:ntw], ez[:, 0:ntw], 1.0)
                    nc.vector.reciprocal(qt[:, nt0:nt0 + ntw], e1[:, 0:ntw])

                def subgather_start(qt, nt0, ntw):
                    """Issue the collective for q t-columns [nt0, nt0+ntw).
                    Returns an emitter for the qc read-back DMA, which the
                    pass loop emits with a 2-chunk lag so the Scalar queue
                    never waits on an in-flight collective."""
                    wsp = gp.tile([128, ntw], f32, tag="wspc")
                    qg = gp.tile([SHARDS, 128, ntw], f32, tag="qgc")
                    nc.scalar.dma_start(out=wsp[:, :], in_=qt[:, nt0:nt0 + ntw])
                    nc.gpsimd.collective_compute(
                        "AllGather", ALU.bypass, replica_groups=RG,
                        ins=[wsp[:].opt()], outs=[qg[:].opt()],
                    )
                    holder = {}

                    def emit_qc():
                        qc = vp.tile([128, SHARDS, NT4], f32, tag=f"qc{nt0 // 4}",
                                     name="qc")
                        nc.scalar.dma_start(out=qc[:, :, 0:ntw],
                                            in_=qg[:, :, :].transpose([1, 0, 2]))
                        holder["qc"] = qc
                    return emit_qc, holder

                def subgather_finish(qc, nt0, ntw, qwg):
                    """DVE half of the gather: w = rsqd*q, split hi/lo f16."""
                    wqc = vp.tile([128, SHARDS, NT4], f32, tag="wqc")
                    nc.vector.tensor_mul(wqc[:, :, 0:ntw],
                                         rsqd[:, :, nt0:nt0 + ntw],
                                         qc[:, :, 0:ntw])
                    nc.vector.tensor_copy(qwg[:, :, 0:ntw, 0],
                                          wqc[:, :, 0:ntw])
                    spl = vp.tile([128, SHARDS, NT4], f32, tag="splc")
                    nc.vector.tensor_sub(spl[:, :, 0:ntw], wqc[:, :, 0:ntw],
                                         qwg[:, :, 0:ntw, 0])
                    nc.vector.tensor_copy(qwg[:, :, 0:ntw, 1],
                                          spl[:, :, 0:ntw])

                def alloc_qw():
                    """One weight tile per gather-group (t//4)."""
                    tiles = []
                    for gi in range(5):
                        qwg = qwp.tile([128, SHARDS, NT4, 2], f16,
                                       tag=f"qw{gi}", name=f"qwg{gi}")
                        tiles.append(qwg)
                    return tiles

                def wsel_of(qws):
                    def wsel(jb, M):
                        g, t = divmod(jb, NT)
                        return qws[t // 4][:, g, t % 4, 0:M]
                    return wsel

                GROUPS = ((0, 4), (4, 4), (8, 4), (12, 4), (16, 2))

                def finish_all(pend, qws):
                    """Emit the f16 weight splits for all gather groups of
                    this pass (feeding the next pass's matmuls)."""
                    for gi, (holder, nt0, ntw) in pend.items():
                        subgather_finish(holder["qc"], nt0, ntw, qws[gi])

                # ---- pass 1: tvec + iteration 1 (M=4) ----
                wsel4 = lambda jb, M: qw4[:, jb, 0:M]
                q1 = vp.tile([128, NT], f32, tag="qpass", name="q1")
                qw_cur = alloc_qw()
                pending = {}

                def fin1(Tsb, nt0, ntw, qt=q1):
                    tv = vp.tile([128, NT], f32, tag="tvc")
                    nc.vector.tensor_add(tv[:, 0:ntw], Tsb[:, nt0:nt0 + ntw, 0],
                                         Tsb[:, nt0:nt0 + ntw, 1])
                    tmpc = vp.tile([128, NT], f32, tag="tmpc")
                    nc.vector.tensor_mul(tmpc[:, 0:ntw], rsqd_l[:, nt0:nt0 + ntw],
                                         tv[:, 0:ntw])
                    tm2c = vp.tile([128, NT], f32, tag="tm2c")
                    nc.vector.tensor_scalar_mul(tm2c[:, 0:ntw], tmpc[:, 0:ntw],
                                                -5.0)
                    nc.vector.tensor_add(Ccoef[:, nt0:nt0 + ntw],
                                         logitU_sb[:, nt0:nt0 + ntw],
                                         tm2c[:, 0:ntw])
                    chunk_z(Tsb, nt0, ntw, qt, 2)
                    emit_qc, holder = subgather_start(qt, nt0, ntw)
                    pending[nt0 // 4] = (holder, nt0, ntw)
                    return emit_qc

                matvec_pass(4, wsel4, fin1, prow=0)
                finish_all(pending, qw_cur)

                # ---- passes 2..11 ----
                for it in range(1, REFINE_ITERS):
                    last = (it == REFINE_ITERS - 1)
                    wsel2 = wsel_of(qw_cur)
                    qt = vp.tile([128, NT], f32, tag="qpass", name="qt")
                    if not last:
                        qw_cur = alloc_qw()
                    pending = {}

                    def fin(Tsb, nt0, ntw, qt=qt, last=last):
                        chunk_z(Tsb, nt0, ntw, qt, 0)
                        if last:
                            nc.sync.dma_start(out=q_out_d[:, nt0:nt0 + ntw],
                                              in_=qt[:, nt0:nt0 + ntw])
                            return None
                        emit_qc, holder = subgather_start(qt, nt0, ntw)
                        pending[nt0 // 4] = (holder, nt0, ntw)
                        return emit_qc

                    matvec_pass(2, wsel2, fin, prow=(32 if it % 2 else 0))
                    if not last:
                        finish_all(pending, qw_cur)

    nc.compile()
    return nc


_NC_CACHE = None


def kernel(imgs, masks):
    global _NC_CACHE
    from concourse.bass_utils import run_bass_kernel_spmd

    in_maps = make_in_maps(imgs, masks)
    if _NC_CACHE is None:
        _NC_CACHE = build_program()
    res = run_bass_kernel_spmd(_NC_CACHE, in_maps, list(range(N_CORES)))
    return assemble(res.results)


# revision 10
# speedup vs baseline: 1.0040x; 1.0040x over previous
"""Trainium2 Bass kernel v3 for CRFHead (dense-Gaussian mean-field CRF).

v2 materialized E = exp(-0.5 d2) once per core in f16 (34/72 j-blocks
SBUF-resident, 38 streamed via DRAM) and ran 11 TensorE matvec passes with
PSUM accumulating across 72 j-block matmuls.  v3 keeps that skeleton and
removes the scheduling stalls found in the v2 trace:

- ~11us inter-pass bubbles: the next pass's first matmul chained through
  [group-4 collective -> qc DMA -> counting-sem -> group-0 weight split].
  v3 emits the qc DMAs with a 2-chunk lag on the Scalar queue and the
  f16 weight splits at the END of the producing pass, so pass k+1's
  weights are ready before pass k's last matmul retires.
- PSUM WAW serialization at pass boundaries: Rps is one [34, ROWS] tile;
  odd passes accumulate into rows 0:M, even passes into rows 32:32+M, so
  the first matmuls of pass k+1 don't wait for pass k's PSUM readers.
- ACT table ping-pong (Ln/Exp per chunk in pass 1's tail, 1.28us per
  reload on the Scalar queue): the per-i 1/sqrt(deg) coefficients now
  come from the exact AllReduced deg via a host-provided shard mask
  (masked reduce over the SHARDS axis) instead of riding pass 1 as a
  ones-weight column; pass 1 shrinks to M=4 and uses only Exp.

Sharding: 8 cores = 2 images x 4-way split of the 9216 output pixels.
"""

import numpy as np

B, C, H, W = 2, 3, 96, 96
N = H * W                 # 9216 pixels
N_CORES = 8
SHARDS = 4                # cores per image
ROWS = N // SHARDS        # 2304 local output rows per core
NT = ROWS // 128          # 18 local 128-row tiles
JB = N // 128             # 72 global j-blocks
NRES = 34                 # j-blocks resident in SBUF (slots NSTREAM..71)
NSTREAM = JB - NRES       # j-blocks streamed from DRAM (slots 0..NSTREAM-1)
KDIM = 12
REFINE_ITERS = 10
RG = [[0, 1, 2, 3], [4, 5, 6, 7]]
ICHUNKS = [(0, 512), (512, 512), (1024, 512), (1536, 512), (2048, 256)]
BCHUNKS = [(0, 1152), (1152, 1152)]
BSUB = [(0, 512), (512, 512), (1024, 128)]
RSTAG = 16                # resident slots reserved for the chunk-major tail
STAG_PICK = (0, 4, 5, 5, 2)
NT4 = 4                   # max t-tiles per i-chunk

F16 = np.float16


def _f16(x):
    return np.asarray(x, dtype=F16).astype(np.float32)


def _split3_f16(w):
    w = np.asarray(w, np.float32)
    w1 = np.asarray(w, F16)
    d1 = w - w1.astype(np.float32)
    w2 = np.asarray(d1, F16)
    w3 = np.asarray(d1 - w2.astype(np.float32), F16)
    return w1, w2, w3


def _host_prep(imgs, masks):
    """Mirror the reference's quantization exactly in numpy fp32."""
    imgs = np.asarray(imgs, np.float32)
    masks = np.asarray(masks, np.float32)
    MEAN = np.array([0.485, 0.456, 0.406], np.float32)[None, :, None, None]
    STD = np.array([0.229, 0.224, 0.225], np.float32)[None, :, None, None]
    x = (imgs * STD + MEAN).transpose(0, 2, 3, 1) * np.float32(255.0)
    x = np.floor(np.clip(x, 0.0, 255.0))
    m = np.floor(np.clip(masks * np.float32(255.0) / np.float32(0.7), 0.0, 255.0))
    return x, m


def _image_data(img_q, mask_q):
    """Per-image full-N host arrays (global row order = row-major pixels)."""
    U = mask_q / (mask_q.max() + 1e-8)
    U = np.clip(U, 1e-6, 1.0 - 1e-6).reshape(N).astype(np.float32)
    logitU = np.log(U / (np.float32(1.0) - U)).astype(np.float32)

    ys, xs = np.meshgrid(np.arange(H, dtype=np.float32),
                         np.arange(W, dtype=np.float32), indexing="ij")
    xv = xs.reshape(N)
    yv = ys.reshape(N)
    c = img_q.reshape(N, 3).astype(np.float32)

    ax = (xv / np.float32(12.0)).astype(np.float32)
    ay = (yv / np.float32(12.0)).astype(np.float32)
    axh = _f16(ax); axl = _f16(ax - axh)
    ayh = _f16(ay); ayl = _f16(ay - ayh)
    r, g, b = _f16(c[:, 0]), _f16(c[:, 1]), _f16(c[:, 2])
    twos = np.full(N, 2.0, np.float32)

    sqxy = xv * xv + yv * yv
    sqrgb = (c * c).sum(axis=1)
    bias = (-sqrgb / np.float32(50.0) - sqxy / np.float32(7200.0)).astype(np.float32)
    whalf = (np.float32(12.5) * bias).astype(np.float32)   # w_i/2; ones-row = 2

    # stationary j-side rows x moving i-side rows -> PSUM holds
    # 25*(f_i . f_j) + w_i ; ACT applies scale 1/25 and per-partition bias_j
    j_rows = np.stack([r, g, b, axh, axh, axl, ayh, ayh, ayl,
                       twos, twos, twos]).astype(F16)           # [12, N]
    i_rows = np.stack([r, g, b, axh, axl, axh, ayh, ayl, ayh,
                       *_split3_f16(whalf)]).astype(F16)        # [12, N]
    return dict(U=U, logitU=logitU, bias=bias,
                j_rows=j_rows, i_rows=i_rows)


def _pb_index():
    """Global row index for [p, jb] layouts: j = (jb//NT)*ROWS + (jb%NT)*128 + p."""
    p = np.arange(128)[:, None]
    jb = np.arange(JB)[None, :]
    return (jb // NT) * ROWS + (jb % NT) * 128 + p      # [128, 72]


def _core_inputs(data, g):
    gidx = _pb_index()
    jlhsT = data["j_rows"][:, gidx.T.reshape(-1)]                  # [12, 72*128]
    isl = slice(g * ROWS, (g + 1) * ROWS)
    irhs = data["i_rows"][:, isl]                                  # [12, 2304]
    biasJ = np.ascontiguousarray(data["bias"][gidx], np.float32)   # [128, 72]
    Ufull = np.ascontiguousarray(data["U"][gidx], np.float32)      # [128, 72]
    lidx = gidx[:, g * NT:(g + 1) * NT]
    logitU = np.ascontiguousarray(data["logitU"][lidx], np.float32)  # [128, 18]
    ident = np.eye(8, dtype=np.float32)
    # one-hot shard mask, [128, SHARDS, NT] flattened as [128, JB]
    maskexp = np.zeros((128, SHARDS, NT), np.float32)
    maskexp[:, g, :] = 1.0
    return {
        "jlhsT": np.ascontiguousarray(jlhsT),
        "irhs": np.ascontiguousarray(irhs),
        "biasJ": biasJ,
        "Ufull": Ufull,
        "logitUl": logitU,
        "ident": ident,
        "maskexp": np.ascontiguousarray(maskexp),
    }


def make_in_maps(imgs, masks):
    x, m = _host_prep(imgs, masks)
    per_image = [_image_data(x[b], m[b]) for b in range(B)]
    in_maps = []
    for k in range(N_CORES):
        b, g = divmod(k, SHARDS)
        in_maps.append(_core_inputs(per_image[b], g))
    return in_maps


def assemble(results):
    out = np.empty((B, N), np.float32)
    p = np.arange(128)[:, None]
    t = np.arange(NT)[None, :]
    lidx = (t * 128 + p).reshape(-1)
    for k in range(N_CORES):
        b, g = divmod(k, SHARDS)
        flat = np.empty(ROWS, np.float32)
        flat[lidx] = np.asarray(results[k]["q_out"], np.float32).reshape(-1)
        out[b, g * ROWS:(g + 1) * ROWS] = flat
    return out.reshape(B, H, W)


def build_program():
    import concourse.bacc as bacc
    import concourse.mybir as mybir
    from concourse.tile import TileContext

    f32 = mybir.dt.float32
    f16 = mybir.dt.float16
    AF = mybir.ActivationFunctionType
    AX = mybir.AxisListType
    ALU = mybir.AluOpType

    nc = bacc.Bacc(num_devices=N_CORES)

    jlhsT_in = nc.dram_tensor("jlhsT", [KDIM, N], f16, kind="ExternalInput")
    irhs_in = nc.dram_tensor("irhs", [KDIM, ROWS], f16, kind="ExternalInput")
    biasJ_in = nc.dram_tensor("biasJ", [128, JB], f32, kind="ExternalInput")
    Ufull_in = nc.dram_tensor("Ufull", [128, JB], f32, kind="ExternalInput")
    logitU_in = nc.dram_tensor("logitUl", [128, NT], f32, kind="ExternalInput")
    ident_in = nc.dram_tensor("ident", [8, 8], f32, kind="ExternalInput")
    maskexp_in = nc.dram_tensor("maskexp", [128, SHARDS, NT], f32,
                                kind="ExternalInput")
    q_out_d = nc.dram_tensor("q_out", [128, NT], f32, kind="ExternalOutput")

    SCALE = float(np.float32(1.0) / np.float32(25.0))

    with TileContext(nc) as tc:
        with (
            tc.tile_pool(name="const", bufs=1) as cpool,
            tc.tile_pool(name="vec", bufs=2) as vp,
            tc.tile_pool(name="ring", bufs=2) as rr,
            tc.tile_pool(name="bounce", bufs=4) as bpl,
            tc.tile_pool(name="sdram", bufs=1, space="DRAM") as dpc,
            tc.tile_pool(name="dramit", bufs=2, space="DRAM") as dp,
            tc.tile_pool(name="gdram", bufs=5, space="DRAM") as gp,
        ):
            # ---- persistent SBUF ----
            jlhsT_sb = cpool.tile([KDIM, N], f16, tag="jlhsT")
            irhs_sb = cpool.tile([KDIM, ROWS], f16, tag="irhs")
            biasJ_sb = cpool.tile([128, JB], f32, tag="biasJ")
            Ufull_sb = cpool.tile([128, JB], f32, tag="Ufull")
            logitU_sb = cpool.tile([128, NT], f32, tag="logitU")
            ident_sb = cpool.tile([8, 8], f32, tag="ident")
            maskexp_sb = cpool.tile([128, SHARDS, NT], f32, tag="maskexp")
            S16 = cpool.tile([128, NRES * ROWS], f16, tag="S16")
            pacc = cpool.tile([128, JB, 2], f32, tag="pacc")
            deg_sb = cpool.tile([128, JB], f32, tag="deg")
            rsqd = cpool.tile([128, SHARDS, NT], f32, tag="rsqd")
            qw4 = cpool.tile([128, JB, 4], f16, tag="qw4")
            rsqd_l = cpool.tile([128, NT], f32, tag="rsqdl")
            Acoef = cpool.tile([128, NT], f32, tag="Acoef")
            Ccoef = cpool.tile([128, NT], f32, tag="Ccoef")

            S_dram = dpc.tile([NSTREAM, 128, ROWS], f16, tag="Sdram")

            nc.sync.dma_start(out=jlhsT_sb[:, :], in_=jlhsT_in[:, :])
            nc.sync.dma_start(out=irhs_sb[:, :], in_=irhs_in[:, :])
            nc.sync.dma_start(out=biasJ_sb[:, :], in_=biasJ_in[:, :])
            nc.sync.dma_start(out=Ufull_sb[:, :], in_=Ufull_in[:, :])
            nc.sync.dma_start(out=logitU_sb[:, :], in_=logitU_in[:, :])
            nc.sync.dma_start(out=ident_sb[:, :], in_=ident_in[:, :])
            nc.sync.dma_start(out=maskexp_sb[:, :, :], in_=maskexp_in[:, :, :])

            # warm up the collective path (first CC op pays ~30us of ring
            # init) while the build matmuls run
            ccw = dp.tile([128, 1], f32, tag="ccw")
            ccwo = dp.tile([SHARDS, 128, 1], f32, tag="ccwo")
            nc.sync.dma_start(out=ccw[:, :], in_=biasJ_sb[:, 0:1])
            nc.gpsimd.collective_compute(
                "AllGather", ALU.bypass, replica_groups=RG,
                ins=[ccw[:].opt()], outs=[ccwo[:].opt()],
            )

            # ================= build pass =================
            with tc.tile_pool(name="bpsum", bufs=2, space="PSUM") as bpp:
                for jb in range(JB):
                    resident = jb >= NSTREAM
                    lhs = jlhsT_sb[:, jb * 128:(jb + 1) * 128]
                    bt = None
                    if not resident:
                        bt = bpl.tile([128, ROWS], f16, tag="bounce")
                    for ci, (c0, cw) in enumerate(BCHUNKS):
                        ps = bpp.tile([128, 1152], f32, tag="bps")
                        for s0, sw in BSUB:
                            nc.tensor.matmul(
                                ps[:, s0:s0 + sw], lhs,
                                irhs_sb[:, c0 + s0:c0 + s0 + sw],
                                start=True, stop=True,
                            )
                        if resident:
                            base = (jb - NSTREAM) * ROWS + c0
                            dst = S16[:, base:base + cw]
                        else:
                            dst = bt[:, c0:c0 + cw]
                        nc.scalar.activation(
                            dst, ps[:, :cw], AF.Exp,
                            bias=biasJ_sb[:, jb:jb + 1], scale=SCALE,
                            accum_out=pacc[:, jb, ci:ci + 1],
                        )
                    if not resident:
                        nc.sync.dma_start(out=S_dram[jb, :, :], in_=bt[:, :])

            # ---- deg: local partials + 4-core AllReduce ----
            pdeg = vp.tile([128, JB], f32, tag="pdeg")
            nc.vector.tensor_reduce(pdeg[:, :], pacc[:, :, :], AX.X, ALU.add)
            pd_d = dp.tile([128, JB], f32, tag="pd")
            pd_o = dp.tile([128, JB], f32, tag="pdo")
            nc.sync.dma_start(out=pd_d[:, :], in_=pdeg[:, :])
            nc.gpsimd.collective_compute(
                "AllReduce", ALU.add, replica_groups=RG,
                ins=[pd_d[:].opt()], outs=[pd_o[:].opt()],
            )
            nc.sync.dma_start(out=deg_sb[:, :], in_=pd_o[:, :])

            ln_deg = vp.tile([128, JB], f32, tag="lndeg")
            nc.scalar.activation(ln_deg[:, :], deg_sb[:, :], AF.Ln)
            nc.scalar.activation(rsqd[:, :, :], ln_deg[:, :], AF.Exp, scale=-0.5)

            # per-i coefficients from the exact deg: select this core's
            # shard columns via the host-provided one-hot mask
            masked = vp.tile([128, SHARDS, NT], f32, tag="maskd")
            nc.vector.tensor_mul(masked[:, :, :], rsqd[:, :, :],
                                 maskexp_sb[:, :, :])
            nc.vector.tensor_reduce(rsqd_l[:, :],
                                    masked[:, :, :].transpose([0, 2, 1]),
                                    AX.X, ALU.add)
            nc.vector.tensor_scalar_mul(Acoef[:, :], rsqd_l[:, :], 10.0)

            def split_hi_lo(src_f32, dst_hi, dst_lo):
                """dst_hi = f16(src); dst_lo = f16(src - dst_hi); dsts strided."""
                nc.vector.tensor_copy(dst_hi, src_f32)
                lo = vp.tile([128, JB], f32, tag="splo")
                nc.vector.tensor_sub(lo[:, :], src_f32, dst_hi)
                nc.vector.tensor_copy(dst_lo, lo[:, :])

            # pass-1 weights: [rsqd_hi, rsqd_lo, (rsqd*U)_hi, (rsqd*U)_lo]
            wq0 = vp.tile([128, JB], f32, tag="wq0")
            nc.vector.tensor_mul(wq0[:, :], rsqd[:, :, :], Ufull_sb[:, :])
            split_hi_lo(rsqd[:, :, :], qw4[:, :, 0], qw4[:, :, 1])
            split_hi_lo(wq0[:, :], qw4[:, :, 2], qw4[:, :, 3])

            # Slot schedule: main region ordered by gather-group (t//4);
            # within a group, streamed/resident slots merge proportionally so
            # the stream DMA hides under resident compute.  The last RSTAG
            # resident slots form the chunk-major stagger tail.
            def slot_schedule():
                res = list(range(NSTREAM, JB))
                stag = []
                for tg in range(5):
                    cand = [jb for jb in res
                            if tg * 4 <= (jb % NT) < min(tg * 4 + 4, NT)]
                    cand.sort(key=lambda jb: -(jb % NT))
                    stag.extend(cand[:STAG_PICK[tg]])
                assert len(stag) == RSTAG
                sset = set(stag)
                main = []
                for t0, tw in ((0, 4), (4, 4), (8, 4), (12, 4), (16, 2)):
                    grp = [jb for jb in range(JB)
                           if t0 <= (jb % NT) < t0 + tw and jb not in sset]
                    S = [jb for jb in grp if jb < NSTREAM]
                    R = [jb for jb in grp if jb >= NSTREAM]
                    ns, nr = len(S), len(R)
                    i = j = 0
                    while i < ns or j < nr:
                        if i < ns and (j >= nr or i * (nr + 1) <= j * (ns + 1)):
                            main.append(S[i])
                            i += 1
                        elif j < nr:
                            main.append(R[j])
                            j += 1
                return main, stag

            # ================= matvec passes =================
            with (
                tc.tile_pool(name="rpsum", bufs=1, space="PSUM") as rpp,
                tc.tile_pool(name="tpsum", bufs=2, space="PSUM") as tpp,
                tc.tile_pool(name="qwp", bufs=2) as qwp,
            ):
                # one PSUM tile; odd passes use rows 0:M, even rows 32:32+M
                Rps_all = rpp.tile([34, ROWS], f32, tag="rps", name="RpsAll")

                def matvec_pass(M, wsel, finish_chunk, prow):
                    """R[0:M] accumulated over all 72 j-blocks into PSUM rows
                    [prow, prow+M); transposed result lands in Tsb
                    [128, NT, :M]; finish_chunk(Tsb, nt0, ntw) runs staggered
                    per chunk and may return a deferred emitter (the gather
                    read-back + f16 weight split), which is emitted with a
                    2-chunk lag so it only waits on its own collective
                    (~6.5us completion latency) and the next pass's first
                    matmuls see their weights ready before this pass ends."""
                    Rps = Rps_all[prow:prow + M, :]
                    Tps = tpp.tile([128, NT, 4], f32, tag="tps", name="Tps")
                    Tsb = vp.tile([128, NT, 4], f32, tag="tsb", name="Tsb")
                    main, stag = slot_schedule()
                    first = True
                    for jb in main:
                        if jb < NSTREAM:
                            st = bpl.tile([128, ROWS], f16, tag="bounce")
                            nc.sync.dma_start(out=st[:, :], in_=S_dram[jb, :, :])
                            src, base0 = st, 0
                        else:
                            src, base0 = S16, (jb - NSTREAM) * ROWS
                        for c0, cw in ICHUNKS:
                            nc.tensor.matmul(
                                Rps[0:M, c0:c0 + cw], wsel(jb, M),
                                src[:, base0 + c0:base0 + c0 + cw],
                                start=first, stop=False,
                            )
                        first = False
                    deferred = []
                    for ci, (c0, cw) in enumerate(ICHUNKS):
                        for si, jb in enumerate(stag):
                            base = (jb - NSTREAM) * ROWS + c0
                            nc.tensor.matmul(
                                Rps[0:M, c0:c0 + cw], wsel(jb, M),
                                S16[:, base:base + cw],
                                start=False, stop=(si == RSTAG - 1),
                            )
                        rg = rr.tile([4, 512], f32, tag="rring")
                        nc.vector.tensor_copy(rg[0:M, 0:cw], Rps[0:M, c0:c0 + cw])
                        nt0, ntw = c0 // 128, cw // 128
                        for tt in range(ntw):
                            nc.tensor.transpose(
                                Tps[:, nt0 + tt, 0:M],
                                rg[0:M, tt * 128:(tt + 1) * 128],
                                ident_sb[0:M, 0:M],
                            )
                        nc.vector.tensor_copy(Tsb[:, nt0:nt0 + ntw, 0:M],
                                              Tps[:, nt0:nt0 + ntw, 0:M])
                        if finish_chunk is not None:
                            d = finish_chunk(Tsb, nt0, ntw)
                            if d is not None:
                                deferred.append(d)
                        if ci >= 2 and len(deferred) > ci - 2:
                            deferred[ci - 2]()
                            deferred[ci - 2] = lambda: None
                    for d in deferred:
                        d()
                    return Tsb

                def chunk_z(Tsb, nt0, ntw, qt, m0):
                    """z = Ccoef + Acoef * (T[m0]+T[m0+1]);
                    q slice <- 1/(1+exp(-z)) (keeps ACT on the Exp set)."""
                    Rr = vp.tile([128, NT], f32, tag="Rrc")
                    nc.vector.tensor_add(Rr[:, 0:ntw], Tsb[:, nt0:nt0 + ntw, m0],
                                         Tsb[:, nt0:nt0 + ntw, m0 + 1])
                    t2 = vp.tile([128, NT], f32, tag="t2c")
                    nc.vector.tensor_mul(t2[:, 0:ntw], Acoef[:, nt0:nt0 + ntw],
                                         Rr[:, 0:ntw])
                    z = vp.tile([128, NT], f32, tag="zc")
                    nc.vector.tensor_add(z[:, 0:ntw], Ccoef[:, nt0:nt0 + ntw],
                                         t2[:, 0:ntw])
                    ez = vp.tile([128, NT], f32, tag="ezc")
                    nc.scalar.activation(ez[:, 0:ntw], z[:, 0:ntw],
                                         AF.Exp, scale=-1.0)
                    e1 = vp.tile([128, NT], f32, tag="e1c")
                    nc.vector.tensor_scalar_add(e1[:, 0:ntw], ez[:, 0:ntw], 1.0)
                    nc.vector.reciprocal(qt[:, nt0:nt0 + ntw], e1[:, 0:ntw])

                def subgather_start(qt, nt0, ntw, qwg):
                    """Issue the collective for q t-columns [nt0, nt0+ntw).
                    Returns an emitter for the qc read-back DMA + f16 weight
                    split into qwg (the next pass's weights)."""
                    wsp = gp.tile([128, ntw], f32, tag="wspc")
                    qg = gp.tile([SHARDS, 128, ntw], f32, tag="qgc")
                    nc.scalar.dma_start(out=wsp[:, :], in_=qt[:, nt0:nt0 + ntw])
                    nc.gpsimd.collective_compute(
                        "AllGather", ALU.bypass, replica_groups=RG,
                        ins=[wsp[:].opt()], outs=[qg[:].opt()],
                    )

                    def emit_tail():
                        qc = vp.tile([128, SHARDS, NT4], f32, tag=f"qc{nt0 // 4}",
                                     name="qc")
                        nc.scalar.dma_start(out=qc[:, :, 0:ntw],
                                            in_=qg[:, :, :].transpose([1, 0, 2]))
                        subgather_finish(qc, nt0, ntw, qwg)
                    return emit_tail

                def subgather_finish(qc, nt0, ntw, qwg):
                    """DVE half of the gather: w = rsqd*q, split hi/lo f16."""
                    wqc = vp.tile([128, SHARDS, NT4], f32, tag="wqc")
                    nc.vector.tensor_mul(wqc[:, :, 0:ntw],
                                         rsqd[:, :, nt0:nt0 + ntw],
                                         qc[:, :, 0:ntw])
                    nc.vector.tensor_copy(qwg[:, :, 0:ntw, 0],
                                          wqc[:, :, 0:ntw])
                    spl = vp.tile([128, SHARDS, NT4], f32, tag="splc")
                    nc.vector.tensor_sub(spl[:, :, 0:ntw], wqc[:, :, 0:ntw],
                                         qwg[:, :, 0:ntw, 0])
                    nc.vector.tensor_copy(qwg[:, :, 0:ntw, 1],
                                          spl[:, :, 0:ntw])

                def alloc_qw():
                    """One weight tile per gather-group (t//4)."""
                    tiles = []
                    for gi in range(5):
                        qwg = qwp.tile([128, SHARDS, NT4, 2], f16,
                                       tag=f"qw{gi}", name=f"qwg{gi}")
                        tiles.append(qwg)
                    return tiles

                def wsel_of(qws):
                    def wsel(jb, M):
                        g, t = divmod(jb, NT)
                        return qws[t // 4][:, g, t % 4, 0:M]
                    return wsel

                # ---- pass 1: tvec + iteration 1 (M=4) ----
                wsel4 = lambda jb, M: qw4[:, jb, 0:M]
                q1 = vp.tile([128, NT], f32, tag="qpass", name="q1")
                qw_cur = alloc_qw()

                def fin1(Tsb, nt0, ntw, qt=q1, qws=qw_cur):
                    tv = vp.tile([128, NT], f32, tag="tvc")
                    nc.vector.tensor_add(tv[:, 0:ntw], Tsb[:, nt0:nt0 + ntw, 0],
                                         Tsb[:, nt0:nt0 + ntw, 1])
                    tmpc = vp.tile([128, NT], f32, tag="tmpc")
                    nc.vector.tensor_mul(tmpc[:, 0:ntw], rsqd_l[:, nt0:nt0 + ntw],
                                         tv[:, 0:ntw])
                    tm2c = vp.tile([128, NT], f32, tag="tm2c")
                    nc.vector.tensor_scalar_mul(tm2c[:, 0:ntw], tmpc[:, 0:ntw],
                                                -5.0)
                    nc.vector.tensor_add(Ccoef[:, nt0:nt0 + ntw],
                                         logitU_sb[:, nt0:nt0 + ntw],
                                         tm2c[:, 0:ntw])
                    chunk_z(Tsb, nt0, ntw, qt, 2)
                    return subgather_start(qt, nt0, ntw, qws[nt0 // 4])

                matvec_pass(4, wsel4, fin1, prow=0)

                # ---- passes 2..11 ----
                for it in range(1, REFINE_ITERS):
                    last = (it == REFINE_ITERS - 1)
                    wsel2 = wsel_of(qw_cur)
                    qt = vp.tile([128, NT], f32, tag="qpass", name="qt")
                    if not last:
                        qw_cur = alloc_qw()

                    def fin(Tsb, nt0, ntw, qt=qt, last=last,
                            qws=(None if last else qw_cur)):
                        chunk_z(Tsb, nt0, ntw, qt, 0)
                        if last:
                            nc.sync.dma_start(out=q_out_d[:, nt0:nt0 + ntw],
                                              in_=qt[:, nt0:nt0 + ntw])
                            return None
                        return subgather_start(qt, nt0, ntw, qws[nt0 // 4])

                    matvec_pass(2, wsel2, fin, prow=(32 if it % 2 else 0))

    nc.compile()
    return nc


_NC_CACHE = None


def kernel(imgs, masks):
    global _NC_CACHE
    from concourse.bass_utils import run_bass_kernel_spmd

    in_maps = make_in_maps(imgs, masks)
    if _NC_CACHE is None:
        _NC_CACHE = build_program()
    res = run_bass_kernel_spmd(_NC_CACHE, in_maps, list(range(N_CORES)))
    return assemble(res.results)


# revision 12
# speedup vs baseline: 1.0319x; 1.0279x over previous
"""Trainium2 Bass kernel v3 for CRFHead (dense-Gaussian mean-field CRF).

v2 materialized E = exp(-0.5 d2) once per core in f16 (34/72 j-blocks
SBUF-resident, 38 streamed via DRAM) and ran 11 TensorE matvec passes with
PSUM accumulating across 72 j-block matmuls.  v3 keeps that skeleton and
removes the scheduling stalls found in the v2 trace:

- ~11us inter-pass bubbles: the next pass's first matmul chained through
  [group-4 collective -> qc DMA -> counting-sem -> group-0 weight split].
  v3 emits the qc DMAs with a 2-chunk lag on the Scalar queue and the
  f16 weight splits at the END of the producing pass, so pass k+1's
  weights are ready before pass k's last matmul retires.
- PSUM WAW serialization at pass boundaries: Rps is one [34, ROWS] tile;
  odd passes accumulate into rows 0:M, even passes into rows 32:32+M, so
  the first matmuls of pass k+1 don't wait for pass k's PSUM readers.
- ACT table ping-pong (Ln/Exp per chunk in pass 1's tail, 1.28us per
  reload on the Scalar queue): the per-i 1/sqrt(deg) coefficients now
  come from the exact AllReduced deg via a host-provided shard mask
  (masked reduce over the SHARDS axis) instead of riding pass 1 as a
  ones-weight column; pass 1 shrinks to M=4 and uses only Exp.

Sharding: 8 cores = 2 images x 4-way split of the 9216 output pixels.
"""

import numpy as np

B, C, H, W = 2, 3, 96, 96
N = H * W                 # 9216 pixels
N_CORES = 8
SHARDS = 4                # cores per image
ROWS = N // SHARDS        # 2304 local output rows per core
NT = ROWS // 128          # 18 local 128-row tiles
JB = N // 128             # 72 global j-blocks
NRES = 34                 # j-blocks resident in SBUF (slots NSTREAM..71)
NSTREAM = JB - NRES       # j-blocks streamed from DRAM (slots 0..NSTREAM-1)
KDIM = 12
REFINE_ITERS = 10
RG = [[0, 1, 2, 3], [4, 5, 6, 7]]
ICHUNKS = [(0, 512), (512, 512), (1024, 512), (1536, 512), (2048, 256)]
BCHUNKS = [(0, 1152), (1152, 1152)]
BSUB = [(0, 512), (512, 512), (1024, 128)]
RSTAG = 16                # resident slots reserved for the chunk-major tail
STAG_PICK = (0, 4, 5, 5, 2)
NT4 = 4                   # max t-tiles per i-chunk

F16 = np.float16


def _f16(x):
    return np.asarray(x, dtype=F16).astype(np.float32)


def _split3_f16(w):
    w = np.asarray(w, np.float32)
    w1 = np.asarray(w, F16)
    d1 = w - w1.astype(np.float32)
    w2 = np.asarray(d1, F16)
    w3 = np.asarray(d1 - w2.astype(np.float32), F16)
    return w1, w2, w3


def _host_prep(imgs, masks):
    """Mirror the reference's quantization exactly in numpy fp32."""
    imgs = np.asarray(imgs, np.float32)
    masks = np.asarray(masks, np.float32)
    MEAN = np.array([0.485, 0.456, 0.406], np.float32)[None, :, None, None]
    STD = np.array([0.229, 0.224, 0.225], np.float32)[None, :, None, None]
    x = (imgs * STD + MEAN).transpose(0, 2, 3, 1) * np.float32(255.0)
    x = np.floor(np.clip(x, 0.0, 255.0))
    m = np.floor(np.clip(masks * np.float32(255.0) / np.float32(0.7), 0.0, 255.0))
    return x, m


def _image_data(img_q, mask_q):
    """Per-image full-N host arrays (global row order = row-major pixels)."""
    U = mask_q / (mask_q.max() + 1e-8)
    U = np.clip(U, 1e-6, 1.0 - 1e-6).reshape(N).astype(np.float32)
    logitU = np.log(U / (np.float32(1.0) - U)).astype(np.float32)

    ys, xs = np.meshgrid(np.arange(H, dtype=np.float32),
                         np.arange(W, dtype=np.float32), indexing="ij")
    xv = xs.reshape(N)
    yv = ys.reshape(N)
    c = img_q.reshape(N, 3).astype(np.float32)

    ax = (xv / np.float32(12.0)).astype(np.float32)
    ay = (yv / np.float32(12.0)).astype(np.float32)
    axh = _f16(ax); axl = _f16(ax - axh)
    ayh = _f16(ay); ayl = _f16(ay - ayh)
    r, g, b = _f16(c[:, 0]), _f16(c[:, 1]), _f16(c[:, 2])
    twos = np.full(N, 2.0, np.float32)

    sqxy = xv * xv + yv * yv
    sqrgb = (c * c).sum(axis=1)
    bias = (-sqrgb / np.float32(50.0) - sqxy / np.float32(7200.0)).astype(np.float32)
    whalf = (np.float32(12.5) * bias).astype(np.float32)   # w_i/2; ones-row = 2

    # stationary j-side rows x moving i-side rows -> PSUM holds
    # 25*(f_i . f_j) + w_i ; ACT applies scale 1/25 and per-partition bias_j
    j_rows = np.stack([r, g, b, axh, axh, axl, ayh, ayh, ayl,
                       twos, twos, twos]).astype(F16)           # [12, N]
    i_rows = np.stack([r, g, b, axh, axl, axh, ayh, ayl, ayh,
                       *_split3_f16(whalf)]).astype(F16)        # [12, N]
    return dict(U=U, logitU=logitU, bias=bias,
                j_rows=j_rows, i_rows=i_rows)


def _pb_index():
    """Global row index for [p, jb] layouts: j = (jb//NT)*ROWS + (jb%NT)*128 + p."""
    p = np.arange(128)[:, None]
    jb = np.arange(JB)[None, :]
    return (jb // NT) * ROWS + (jb % NT) * 128 + p      # [128, 72]


def _core_inputs(data, g):
    gidx = _pb_index()
    jlhsT = data["j_rows"][:, gidx.T.reshape(-1)]                  # [12, 72*128]
    isl = slice(g * ROWS, (g + 1) * ROWS)
    irhs = data["i_rows"][:, isl]                                  # [12, 2304]
    biasJ = np.ascontiguousarray(data["bias"][gidx], np.float32)   # [128, 72]
    Ufull = np.ascontiguousarray(data["U"][gidx], np.float32)      # [128, 72]
    lidx = gidx[:, g * NT:(g + 1) * NT]
    logitU = np.ascontiguousarray(data["logitU"][lidx], np.float32)  # [128, 18]
    ident = np.eye(8, dtype=np.float32)
    # one-hot shard mask, [128, SHARDS, NT] flattened as [128, JB]
    maskexp = np.zeros((128, SHARDS, NT), np.float32)
    maskexp[:, g, :] = 1.0
    return {
        "jlhsT": np.ascontiguousarray(jlhsT),
        "irhs": np.ascontiguousarray(irhs),
        "biasJ": biasJ,
        "Ufull": Ufull,
        "logitUl": logitU,
        "ident": ident,
        "maskexp": np.ascontiguousarray(maskexp),
    }


def make_in_maps(imgs, masks):
    x, m = _host_prep(imgs, masks)
    per_image = [_image_data(x[b], m[b]) for b in range(B)]
    in_maps = []
    for k in range(N_CORES):
        b, g = divmod(k, SHARDS)
        in_maps.append(_core_inputs(per_image[b], g))
    return in_maps


def assemble(results):
    out = np.empty((B, N), np.float32)
    p = np.arange(128)[:, None]
    t = np.arange(NT)[None, :]
    lidx = (t * 128 + p).reshape(-1)
    for k in range(N_CORES):
        b, g = divmod(k, SHARDS)
        flat = np.empty(ROWS, np.float32)
        flat[lidx] = np.asarray(results[k]["q_out"], np.float32).reshape(-1)
        out[b, g * ROWS:(g + 1) * ROWS] = flat
    return out.reshape(B, H, W)


def build_program():
    import concourse.bacc as bacc
    import concourse.mybir as mybir
    from concourse.tile import TileContext

    f32 = mybir.dt.float32
    f16 = mybir.dt.float16
    AF = mybir.ActivationFunctionType
    AX = mybir.AxisListType
    ALU = mybir.AluOpType

    nc = bacc.Bacc(num_devices=N_CORES)

    jlhsT_in = nc.dram_tensor("jlhsT", [KDIM, N], f16, kind="ExternalInput")
    irhs_in = nc.dram_tensor("irhs", [KDIM, ROWS], f16, kind="ExternalInput")
    biasJ_in = nc.dram_tensor("biasJ", [128, JB], f32, kind="ExternalInput")
    Ufull_in = nc.dram_tensor("Ufull", [128, JB], f32, kind="ExternalInput")
    logitU_in = nc.dram_tensor("logitUl", [128, NT], f32, kind="ExternalInput")
    ident_in = nc.dram_tensor("ident", [8, 8], f32, kind="ExternalInput")
    maskexp_in = nc.dram_tensor("maskexp", [128, SHARDS, NT], f32,
                                kind="ExternalInput")
    q_out_d = nc.dram_tensor("q_out", [128, NT], f32, kind="ExternalOutput")

    SCALE = float(np.float32(1.0) / np.float32(25.0))

    with TileContext(nc) as tc:
        with (
            tc.tile_pool(name="const", bufs=1) as cpool,
            tc.tile_pool(name="vec", bufs=2) as vp,
            tc.tile_pool(name="ring", bufs=2) as rr,
            tc.tile_pool(name="bounce", bufs=4) as bpl,
            tc.tile_pool(name="sdram", bufs=1, space="DRAM") as dpc,
            tc.tile_pool(name="dramit", bufs=2, space="DRAM") as dp,
            tc.tile_pool(name="gdram", bufs=5, space="DRAM") as gp,
        ):
            # ---- persistent SBUF ----
            jlhsT_sb = cpool.tile([KDIM, N], f16, tag="jlhsT")
            irhs_sb = cpool.tile([KDIM, ROWS], f16, tag="irhs")
            biasJ_sb = cpool.tile([128, JB], f32, tag="biasJ")
            Ufull_sb = cpool.tile([128, JB], f32, tag="Ufull")
            logitU_sb = cpool.tile([128, NT], f32, tag="logitU")
            ident_sb = cpool.tile([8, 8], f32, tag="ident")
            maskexp_sb = cpool.tile([128, SHARDS, NT], f32, tag="maskexp")
            S16 = cpool.tile([128, NRES * ROWS], f16, tag="S16")
            pacc = cpool.tile([128, JB, 2], f32, tag="pacc")
            deg_sb = cpool.tile([128, JB], f32, tag="deg")
            rsqd = cpool.tile([128, SHARDS, NT], f32, tag="rsqd")
            qw4 = cpool.tile([128, JB, 4], f16, tag="qw4")
            rsqd_l = cpool.tile([128, NT], f32, tag="rsqdl")
            Acoef = cpool.tile([128, NT], f32, tag="Acoef")
            Ccoef = cpool.tile([128, NT], f32, tag="Ccoef")

            S_dram = dpc.tile([NSTREAM, 128, ROWS], f16, tag="Sdram")

            nc.sync.dma_start(out=jlhsT_sb[:, :], in_=jlhsT_in[:, :])
            nc.sync.dma_start(out=irhs_sb[:, :], in_=irhs_in[:, :])
            nc.sync.dma_start(out=biasJ_sb[:, :], in_=biasJ_in[:, :])
            nc.sync.dma_start(out=Ufull_sb[:, :], in_=Ufull_in[:, :])
            nc.sync.dma_start(out=logitU_sb[:, :], in_=logitU_in[:, :])
            nc.sync.dma_start(out=ident_sb[:, :], in_=ident_in[:, :])
            nc.sync.dma_start(out=maskexp_sb[:, :, :], in_=maskexp_in[:, :, :])

            # warm up the collective path (first CC op pays ~30us of ring
            # init) while the build matmuls run
            ccw = dp.tile([128, 1], f32, tag="ccw")
            ccwo = dp.tile([SHARDS, 128, 1], f32, tag="ccwo")
            nc.sync.dma_start(out=ccw[:, :], in_=biasJ_sb[:, 0:1])
            nc.gpsimd.collective_compute(
                "AllGather", ALU.bypass, replica_groups=RG,
                ins=[ccw[:].opt()], outs=[ccwo[:].opt()],
            )

            # ================= build pass =================
            with tc.tile_pool(name="bpsum", bufs=2, space="PSUM") as bpp:
                for jb in range(JB):
                    resident = jb >= NSTREAM
                    lhs = jlhsT_sb[:, jb * 128:(jb + 1) * 128]
                    bt = None
                    if not resident:
                        bt = bpl.tile([128, ROWS], f16, tag="bounce")
                    for ci, (c0, cw) in enumerate(BCHUNKS):
                        ps = bpp.tile([128, 1152], f32, tag="bps")
                        for s0, sw in BSUB:
                            nc.tensor.matmul(
                                ps[:, s0:s0 + sw], lhs,
                                irhs_sb[:, c0 + s0:c0 + s0 + sw],
                                start=True, stop=True,
                            )
                        if resident:
                            base = (jb - NSTREAM) * ROWS + c0
                            dst = S16[:, base:base + cw]
                        else:
                            dst = bt[:, c0:c0 + cw]
                        nc.scalar.activation(
                            dst, ps[:, :cw], AF.Exp,
                            bias=biasJ_sb[:, jb:jb + 1], scale=SCALE,
                            accum_out=pacc[:, jb, ci:ci + 1],
                        )
                    if not resident:
                        nc.sync.dma_start(out=S_dram[jb, :, :], in_=bt[:, :])

            # ---- deg: local partials + 4-core AllReduce ----
            pdeg = vp.tile([128, JB], f32, tag="pdeg")
            nc.vector.tensor_reduce(pdeg[:, :], pacc[:, :, :], AX.X, ALU.add)
            pd_d = dp.tile([128, JB], f32, tag="pd")
            pd_o = dp.tile([128, JB], f32, tag="pdo")
            nc.sync.dma_start(out=pd_d[:, :], in_=pdeg[:, :])
            nc.gpsimd.collective_compute(
                "AllReduce", ALU.add, replica_groups=RG,
                ins=[pd_d[:].opt()], outs=[pd_o[:].opt()],
            )
            nc.sync.dma_start(out=deg_sb[:, :], in_=pd_o[:, :])

            ln_deg = vp.tile([128, JB], f32, tag="lndeg")
            nc.scalar.activation(ln_deg[:, :], deg_sb[:, :], AF.Ln)
            nc.scalar.activation(rsqd[:, :, :], ln_deg[:, :], AF.Exp, scale=-0.5)

            # per-i coefficients from the exact deg: select this core's
            # shard columns via the host-provided one-hot mask
            masked = vp.tile([128, SHARDS, NT], f32, tag="maskd")
            nc.vector.tensor_mul(masked[:, :, :], rsqd[:, :, :],
                                 maskexp_sb[:, :, :])
            nc.vector.tensor_reduce(rsqd_l[:, :],
                                    masked[:, :, :].transpose([0, 2, 1]),
                                    AX.X, ALU.add)
            nc.vector.tensor_scalar_mul(Acoef[:, :], rsqd_l[:, :], 10.0)

            def split_hi_lo(src_f32, dst_hi, dst_lo):
                """dst_hi = f16(src); dst_lo = f16(src - dst_hi); dsts strided."""
                nc.vector.tensor_copy(dst_hi, src_f32)
                lo = vp.tile([128, JB], f32, tag="splo")
                nc.vector.tensor_sub(lo[:, :], src_f32, dst_hi)
                nc.vector.tensor_copy(dst_lo, lo[:, :])

            # pass-1 weights: [rsqd_hi, rsqd_lo, (rsqd*U)_hi, (rsqd*U)_lo]
            wq0 = vp.tile([128, JB], f32, tag="wq0")
            nc.vector.tensor_mul(wq0[:, :], rsqd[:, :, :], Ufull_sb[:, :])
            split_hi_lo(rsqd[:, :, :], qw4[:, :, 0], qw4[:, :, 1])
            split_hi_lo(wq0[:, :], qw4[:, :, 2], qw4[:, :, 3])

            # Slot schedule: main region ordered by gather-group (t//4);
            # within a group, streamed/resident slots merge proportionally so
            # the stream DMA hides under resident compute.  The last RSTAG
            # resident slots form the chunk-major stagger tail.
            def slot_schedule():
                res = list(range(NSTREAM, JB))
                stag = []
                for tg in range(5):
                    cand = [jb for jb in res
                            if tg * 4 <= (jb % NT) < min(tg * 4 + 4, NT)]
                    cand.sort(key=lambda jb: -(jb % NT))
                    stag.extend(cand[:STAG_PICK[tg]])
                assert len(stag) == RSTAG
                sset = set(stag)
                main = []
                for t0, tw in ((0, 4), (4, 4), (8, 4), (12, 4), (16, 2)):
                    grp = [jb for jb in range(JB)
                           if t0 <= (jb % NT) < t0 + tw and jb not in sset]
                    S = [jb for jb in grp if jb < NSTREAM]
                    R = [jb for jb in grp if jb >= NSTREAM]
                    ns, nr = len(S), len(R)
                    i = j = 0
                    # group 0 leads with residents: covers the first streamed
                    # DMAs at a pass boundary
                    lead = 2 if t0 == 0 else 0
                    while j < nr and j < lead:
                        main.append(R[j])
                        j += 1
                    while i < ns or j < nr:
                        if i < ns and (j >= nr or i * (nr + 1) <= j * (ns + 1)):
                            main.append(S[i])
                            i += 1
                        elif j < nr:
                            main.append(R[j])
                            j += 1
                return main, stag

            # ================= matvec passes =================
            with (
                tc.tile_pool(name="rpsum", bufs=1, space="PSUM") as rpp,
                tc.tile_pool(name="tpsum", bufs=2, space="PSUM") as tpp,
                tc.tile_pool(name="qwp", bufs=2) as qwp,
            ):
                # one PSUM tile; odd passes use rows 0:M, even rows 32:32+M
                Rps_all = rpp.tile([34, ROWS], f32, tag="rps", name="RpsAll")

                def matvec_pass(M, wsel, finish_chunk, prow):
                    """R[0:M] accumulated over all 72 j-blocks into PSUM rows
                    [prow, prow+M); transposed result lands in Tsb
                    [128, NT, :M]; finish_chunk(Tsb, nt0, ntw) runs staggered
                    per chunk and may return a deferred emitter (the gather
                    read-back + f16 weight split), which is emitted with a
                    2-chunk lag so it only waits on its own collective
                    (~6.5us completion latency) and the next pass's first
                    matmuls see their weights ready before this pass ends."""
                    Rps = Rps_all[prow:prow + M, :]
                    Tps = tpp.tile([128, NT, 4], f32, tag="tps", name="Tps")
                    Tsb = vp.tile([128, NT, 4], f32, tag="tsb", name="Tsb")
                    main, stag = slot_schedule()
                    first = True
                    for jb in main:
                        if jb < NSTREAM:
                            st = bpl.tile([128, ROWS], f16, tag="bounce")
                            nc.sync.dma_start(out=st[:, :], in_=S_dram[jb, :, :])
                            src, base0 = st, 0
                        else:
                            src, base0 = S16, (jb - NSTREAM) * ROWS
                        for c0, cw in ICHUNKS:
                            nc.tensor.matmul(
                                Rps[0:M, c0:c0 + cw], wsel(jb, M),
                                src[:, base0 + c0:base0 + c0 + cw],
                                start=first, stop=False,
                            )
                        first = False
                    deferred = []
                    for ci, (c0, cw) in enumerate(ICHUNKS):
                        for si, jb in enumerate(stag):
                            base = (jb - NSTREAM) * ROWS + c0
                            nc.tensor.matmul(
                                Rps[0:M, c0:c0 + cw], wsel(jb, M),
                                S16[:, base:base + cw],
                                start=False, stop=(si == RSTAG - 1),
                            )
                        rg = rr.tile([4, 512], f32, tag="rring")
                        nc.vector.tensor_copy(rg[0:M, 0:cw], Rps[0:M, c0:c0 + cw])
                        nt0, ntw = c0 // 128, cw // 128
                        for tt in range(ntw):
                            nc.tensor.transpose(
                                Tps[:, nt0 + tt, 0:M],
                                rg[0:M, tt * 128:(tt + 1) * 128],
                                ident_sb[0:M, 0:M],
                            )
                        nc.vector.tensor_copy(Tsb[:, nt0:nt0 + ntw, 0:M],
                                              Tps[:, nt0:nt0 + ntw, 0:M])
                        if finish_chunk is not None:
                            d = finish_chunk(Tsb, nt0, ntw)
                            if d is not None:
                                deferred.append(d)
                        if ci >= 2 and len(deferred) > ci - 2:
                            deferred[ci - 2]()
                            deferred[ci - 2] = lambda: None
                    for d in deferred:
                        d()
                    return Tsb

                def chunk_z(Tsb, nt0, ntw, qt, m0):
                    """z = Ccoef + Acoef * (T[m0]+T[m0+1]);
                    q slice <- 1/(1+exp(-z)) (keeps ACT on the Exp set)."""
                    Rr = vp.tile([128, NT], f32, tag="Rrc")
                    nc.vector.tensor_add(Rr[:, 0:ntw], Tsb[:, nt0:nt0 + ntw, m0],
                                         Tsb[:, nt0:nt0 + ntw, m0 + 1])
                    t2 = vp.tile([128, NT], f32, tag="t2c")
                    nc.vector.tensor_mul(t2[:, 0:ntw], Acoef[:, nt0:nt0 + ntw],
                                         Rr[:, 0:ntw])
                    z = vp.tile([128, NT], f32, tag="zc")
                    nc.vector.tensor_add(z[:, 0:ntw], Ccoef[:, nt0:nt0 + ntw],
                                         t2[:, 0:ntw])
                    ez = vp.tile([128, NT], f32, tag="ezc")
                    nc.scalar.activation(ez[:, 0:ntw], z[:, 0:ntw],
                                         AF.Exp, scale=-1.0)
                    e1 = vp.tile([128, NT], f32, tag="e1c")
                    nc.vector.tensor_scalar_add(e1[:, 0:ntw], ez[:, 0:ntw], 1.0)
                    nc.vector.reciprocal(qt[:, nt0:nt0 + ntw], e1[:, 0:ntw])

                def subgather_start(qt, nt0, ntw, qwg):
                    """Issue the collective for q t-columns [nt0, nt0+ntw).
                    Returns an emitter for the qc read-back DMA + f16 weight
                    split into qwg (the next pass's weights)."""
                    wsp = gp.tile([128, ntw], f32, tag="wspc")
                    qg = gp.tile([SHARDS, 128, ntw], f32, tag="qgc")
                    nc.scalar.dma_start(out=wsp[:, :], in_=qt[:, nt0:nt0 + ntw])
                    nc.gpsimd.collective_compute(
                        "AllGather", ALU.bypass, replica_groups=RG,
                        ins=[wsp[:].opt()], outs=[qg[:].opt()],
                    )

                    def emit_tail():
                        # qc rides the GpSimd queue: the Tile scheduler
                        # re-orders per-queue streams, and on Scalar/Sync this
                        # DMA's wait on the collective would stall z-exps or
                        # the bounce stream behind it.  The GpSimd queue only
                        # carries collective launches, which are already
                        # serialized by the CC core.
                        qc = vp.tile([128, SHARDS, NT4], f32, tag=f"qc{nt0 // 4}",
                                     name="qc")
                        nc.gpsimd.dma_start(out=qc[:, :, 0:ntw],
                                            in_=qg[:, :, :].transpose([1, 0, 2]))
                        subgather_finish(qc, nt0, ntw, qwg)
                    return emit_tail

                def subgather_finish(qc, nt0, ntw, qwg):
                    """DVE half of the gather: w = rsqd*q, split hi/lo f16."""
                    wqc = vp.tile([128, SHARDS, NT4], f32, tag="wqc")
                    nc.vector.tensor_mul(wqc[:, :, 0:ntw],
                                         rsqd[:, :, nt0:nt0 + ntw],
                                         qc[:, :, 0:ntw])
                    nc.vector.tensor_copy(qwg[:, :, 0:ntw, 0],
                                          wqc[:, :, 0:ntw])
                    spl = vp.tile([128, SHARDS, NT4], f32, tag="splc")
                    nc.vector.tensor_sub(spl[:, :, 0:ntw], wqc[:, :, 0:ntw],
                                         qwg[:, :, 0:ntw, 0])
                    nc.vector.tensor_copy(qwg[:, :, 0:ntw, 1],
                                          spl[:, :, 0:ntw])

                def alloc_qw():
                    """One weight tile per gather-group (t//4)."""
                    tiles = []
                    for gi in range(5):
                        qwg = qwp.tile([128, SHARDS, NT4, 2], f16,
                                       tag=f"qw{gi}", name=f"qwg{gi}")
                        tiles.append(qwg)
                    return tiles

                def wsel_of(qws):
                    def wsel(jb, M):
                        g, t = divmod(jb, NT)
                        return qws[t // 4][:, g, t % 4, 0:M]
                    return wsel

                # ---- pass 1: tvec + iteration 1 (M=4) ----
                wsel4 = lambda jb, M: qw4[:, jb, 0:M]
                q1 = vp.tile([128, NT], f32, tag="qpass", name="q1")
                qw_cur = alloc_qw()

                def fin1(Tsb, nt0, ntw, qt=q1, qws=qw_cur):
                    tv = vp.tile([128, NT], f32, tag="tvc")
                    nc.vector.tensor_add(tv[:, 0:ntw], Tsb[:, nt0:nt0 + ntw, 0],
                                         Tsb[:, nt0:nt0 + ntw, 1])
                    tmpc = vp.tile([128, NT], f32, tag="tmpc")
                    nc.vector.tensor_mul(tmpc[:, 0:ntw], rsqd_l[:, nt0:nt0 + ntw],
                                         tv[:, 0:ntw])
                    tm2c = vp.tile([128, NT], f32, tag="tm2c")
                    nc.vector.tensor_scalar_mul(tm2c[:, 0:ntw], tmpc[:, 0:ntw],
                                                -5.0)
                    nc.vector.tensor_add(Ccoef[:, nt0:nt0 + ntw],
                                         logitU_sb[:, nt0:nt0 + ntw],
                                         tm2c[:, 0:ntw])
                    chunk_z(Tsb, nt0, ntw, qt, 2)
                    return subgather_start(qt, nt0, ntw, qws[nt0 // 4])

                matvec_pass(4, wsel4, fin1, prow=0)

                # ---- passes 2..11 ----
                for it in range(1, REFINE_ITERS):
                    last = (it == REFINE_ITERS - 1)
                    wsel2 = wsel_of(qw_cur)
                    qt = vp.tile([128, NT], f32, tag="qpass", name="qt")
                    if not last:
                        qw_cur = alloc_qw()

                    def fin(Tsb, nt0, ntw, qt=qt, last=last,
                            qws=(None if last else qw_cur)):
                        chunk_z(Tsb, nt0, ntw, qt, 0)
                        if last:
                            nc.sync.dma_start(out=q_out_d[:, nt0:nt0 + ntw],
                                              in_=qt[:, nt0:nt0 + ntw])
                            return None
                        return subgather_start(qt, nt0, ntw, qws[nt0 // 4])

                    matvec_pass(2, wsel2, fin, prow=(32 if it % 2 else 0))

    nc.compile()
    return nc


_NC_CACHE = None


def kernel(imgs, masks):
    global _NC_CACHE
    from concourse.bass_utils import run_bass_kernel_spmd

    in_maps = make_in_maps(imgs, masks)
    if _NC_CACHE is None:
        _NC_CACHE = build_program()
    res = run_bass_kernel_spmd(_NC_CACHE, in_maps, list(range(N_CORES)))
    return assemble(res.results)


# revision 18
# speedup vs baseline: 1.0699x; 1.0368x over previous
"""Trainium2 Bass kernel v3 for CRFHead (dense-Gaussian mean-field CRF).

v2 materialized E = exp(-0.5 d2) once per core in f16 (34/72 j-blocks
SBUF-resident, 38 streamed via DRAM) and ran 11 TensorE matvec passes with
PSUM accumulating across 72 j-block matmuls.  v3 keeps that skeleton and
removes the scheduling stalls found in the v2 trace:

- ~11us inter-pass bubbles: the next pass's first matmul chained through
  [group-4 collective -> qc DMA -> counting-sem -> group-0 weight split].
  v3 emits the qc DMAs with a 2-chunk lag on the Scalar queue and the
  f16 weight splits at the END of the producing pass, so pass k+1's
  weights are ready before pass k's last matmul retires.
- PSUM WAW serialization at pass boundaries: Rps is one [34, ROWS] tile;
  odd passes accumulate into rows 0:M, even passes into rows 32:32+M, so
  the first matmuls of pass k+1 don't wait for pass k's PSUM readers.
- ACT table ping-pong (Ln/Exp per chunk in pass 1's tail, 1.28us per
  reload on the Scalar queue): the per-i 1/sqrt(deg) coefficients now
  come from the exact AllReduced deg via a host-provided shard mask
  (masked reduce over the SHARDS axis) instead of riding pass 1 as a
  ones-weight column; pass 1 shrinks to M=4 and uses only Exp.

Sharding: 8 cores = 2 images x 4-way split of the 9216 output pixels.
"""

import numpy as np

B, C, H, W = 2, 3, 96, 96
N = H * W                 # 9216 pixels
N_CORES = 8
SHARDS = 4                # cores per image
ROWS = N // SHARDS        # 2304 local output rows per core
NT = ROWS // 128          # 18 local 128-row tiles
JB = N // 128             # 72 global j-blocks
NRES = 36                 # j-blocks resident in SBUF (slots NSTREAM..71)
NSTREAM = JB - NRES       # j-blocks streamed from DRAM (slots 0..NSTREAM-1)
KDIM = 12
REFINE_ITERS = 10
RG = [[0, 1, 2, 3], [4, 5, 6, 7]]
ICHUNKS = [(0, 512), (512, 512), (1024, 512), (1536, 512), (2048, 256)]
BCHUNKS = [(0, 1152), (1152, 1152)]
BSUB = [(0, 512), (512, 512), (1024, 128)]
RSTAG = 16                # resident slots reserved for the chunk-major tail
STAG_PICK = (0, 4, 5, 5, 2)
NT4 = 4                   # max t-tiles per i-chunk

F16 = np.float16


def _f16(x):
    return np.asarray(x, dtype=F16).astype(np.float32)


def _split3_f16(w):
    w = np.asarray(w, np.float32)
    w1 = np.asarray(w, F16)
    d1 = w - w1.astype(np.float32)
    w2 = np.asarray(d1, F16)
    w3 = np.asarray(d1 - w2.astype(np.float32), F16)
    return w1, w2, w3


def _host_prep(imgs, masks):
    """Mirror the reference's quantization exactly in numpy fp32."""
    imgs = np.asarray(imgs, np.float32)
    masks = np.asarray(masks, np.float32)
    MEAN = np.array([0.485, 0.456, 0.406], np.float32)[None, :, None, None]
    STD = np.array([0.229, 0.224, 0.225], np.float32)[None, :, None, None]
    x = (imgs * STD + MEAN).transpose(0, 2, 3, 1) * np.float32(255.0)
    x = np.floor(np.clip(x, 0.0, 255.0))
    m = np.floor(np.clip(masks * np.float32(255.0) / np.float32(0.7), 0.0, 255.0))
    return x, m


def _image_data(img_q, mask_q):
    """Per-image full-N host arrays (global row order = row-major pixels)."""
    U = mask_q / (mask_q.max() + 1e-8)
    U = np.clip(U, 1e-6, 1.0 - 1e-6).reshape(N).astype(np.float32)
    logitU = np.log(U / (np.float32(1.0) - U)).astype(np.float32)

    ys, xs = np.meshgrid(np.arange(H, dtype=np.float32),
                         np.arange(W, dtype=np.float32), indexing="ij")
    xv = xs.reshape(N)
    yv = ys.reshape(N)
    c = img_q.reshape(N, 3).astype(np.float32)

    ax = (xv / np.float32(12.0)).astype(np.float32)
    ay = (yv / np.float32(12.0)).astype(np.float32)
    axh = _f16(ax); axl = _f16(ax - axh)
    ayh = _f16(ay); ayl = _f16(ay - ayh)
    r, g, b = _f16(c[:, 0]), _f16(c[:, 1]), _f16(c[:, 2])
    twos = np.full(N, 2.0, np.float32)

    sqxy = xv * xv + yv * yv
    sqrgb = (c * c).sum(axis=1)
    bias = (-sqrgb / np.float32(50.0) - sqxy / np.float32(7200.0)).astype(np.float32)
    whalf = (np.float32(12.5) * bias).astype(np.float32)   # w_i/2; ones-row = 2

    # stationary j-side rows x moving i-side rows -> PSUM holds
    # 25*(f_i . f_j) + w_i ; ACT applies scale 1/25 and per-partition bias_j
    j_rows = np.stack([r, g, b, axh, axh, axl, ayh, ayh, ayl,
                       twos, twos, twos]).astype(F16)           # [12, N]
    i_rows = np.stack([r, g, b, axh, axl, axh, ayh, ayl, ayh,
                       *_split3_f16(whalf)]).astype(F16)        # [12, N]
    return dict(U=U, logitU=logitU, bias=bias,
                j_rows=j_rows, i_rows=i_rows)


def _pb_index():
    """Global row index for [p, jb] layouts: j = (jb//NT)*ROWS + (jb%NT)*128 + p."""
    p = np.arange(128)[:, None]
    jb = np.arange(JB)[None, :]
    return (jb // NT) * ROWS + (jb % NT) * 128 + p      # [128, 72]


def _core_inputs(data, g):
    gidx = _pb_index()
    jlhsT = data["j_rows"][:, gidx.T.reshape(-1)]                  # [12, 72*128]
    isl = slice(g * ROWS, (g + 1) * ROWS)
    irhs = data["i_rows"][:, isl]                                  # [12, 2304]
    biasJ = np.ascontiguousarray(data["bias"][gidx], np.float32)   # [128, 72]
    Ufull = np.ascontiguousarray(data["U"][gidx], np.float32)      # [128, 72]
    lidx = gidx[:, g * NT:(g + 1) * NT]
    logitU = np.ascontiguousarray(data["logitU"][lidx], np.float32)  # [128, 18]
    ident = np.eye(8, dtype=np.float32)
    # one-hot shard mask, [128, SHARDS, NT] flattened as [128, JB]
    maskexp = np.zeros((128, SHARDS, NT), np.float32)
    maskexp[:, g, :] = 1.0
    return {
        "jlhsT": np.ascontiguousarray(jlhsT),
        "irhs": np.ascontiguousarray(irhs),
        "biasJ": biasJ,
        "Ufull": Ufull,
        "logitUl": logitU,
        "ident": ident,
        "maskexp": np.ascontiguousarray(maskexp),
    }


def make_in_maps(imgs, masks):
    x, m = _host_prep(imgs, masks)
    per_image = [_image_data(x[b], m[b]) for b in range(B)]
    in_maps = []
    for k in range(N_CORES):
        b, g = divmod(k, SHARDS)
        in_maps.append(_core_inputs(per_image[b], g))
    return in_maps


def assemble(results):
    out = np.empty((B, N), np.float32)
    p = np.arange(128)[:, None]
    t = np.arange(NT)[None, :]
    lidx = (t * 128 + p).reshape(-1)
    for k in range(N_CORES):
        b, g = divmod(k, SHARDS)
        flat = np.empty(ROWS, np.float32)
        flat[lidx] = np.asarray(results[k]["q_out"], np.float32).reshape(-1)
        out[b, g * ROWS:(g + 1) * ROWS] = flat
    return out.reshape(B, H, W)


def build_program():
    import concourse.bacc as bacc
    import concourse.mybir as mybir
    from concourse.tile import TileContext

    f32 = mybir.dt.float32
    f16 = mybir.dt.float16
    AF = mybir.ActivationFunctionType
    AX = mybir.AxisListType
    ALU = mybir.AluOpType

    nc = bacc.Bacc(num_devices=N_CORES)

    jlhsT_in = nc.dram_tensor("jlhsT", [KDIM, N], f16, kind="ExternalInput")
    irhs_in = nc.dram_tensor("irhs", [KDIM, ROWS], f16, kind="ExternalInput")
    biasJ_in = nc.dram_tensor("biasJ", [128, JB], f32, kind="ExternalInput")
    Ufull_in = nc.dram_tensor("Ufull", [128, JB], f32, kind="ExternalInput")
    logitU_in = nc.dram_tensor("logitUl", [128, NT], f32, kind="ExternalInput")
    ident_in = nc.dram_tensor("ident", [8, 8], f32, kind="ExternalInput")
    maskexp_in = nc.dram_tensor("maskexp", [128, SHARDS, NT], f32,
                                kind="ExternalInput")
    q_out_d = nc.dram_tensor("q_out", [128, NT], f32, kind="ExternalOutput")

    SCALE = float(np.float32(1.0) / np.float32(25.0))

    with TileContext(nc) as tc:
        with (
            tc.tile_pool(name="const", bufs=1) as cpool,
            tc.tile_pool(name="vec", bufs=2) as vp,
            tc.tile_pool(name="ring", bufs=2) as rr,
            tc.tile_pool(name="sdram", bufs=1, space="DRAM") as dpc,
            tc.tile_pool(name="dramit", bufs=2, space="DRAM") as dp,
            tc.tile_pool(name="gdram", bufs=5, space="DRAM") as gp,
        ):
            # ---- persistent SBUF ----
            biasJ_sb = cpool.tile([128, JB], f32, tag="biasJ")
            Ufull_sb = cpool.tile([128, JB], f32, tag="Ufull")
            logitU_sb = cpool.tile([128, NT], f32, tag="logitU")
            ident_sb = cpool.tile([8, 8], f32, tag="ident")
            maskexp_sb = cpool.tile([128, SHARDS, NT], f32, tag="maskexp")
            S16 = cpool.tile([128, NRES * ROWS], f16, tag="S16")
            pacc = cpool.tile([128, JB, 2], f32, tag="pacc")
            deg_sb = cpool.tile([128, JB], f32, tag="deg")
            rsqd = cpool.tile([128, SHARDS, NT], f32, tag="rsqd")
            qw4 = cpool.tile([128, JB, 4], f16, tag="qw4")
            rsqd_l = cpool.tile([128, NT], f32, tag="rsqdl")
            Acoef = cpool.tile([128, NT], f32, tag="Acoef")
            Ccoef = cpool.tile([128, NT], f32, tag="Ccoef")

            S_dram = dpc.tile([NSTREAM, 128, ROWS], f16, tag="Sdram")

            nc.sync.dma_start(out=biasJ_sb[:, :], in_=biasJ_in[:, :])
            nc.sync.dma_start(out=Ufull_sb[:, :], in_=Ufull_in[:, :])
            nc.sync.dma_start(out=logitU_sb[:, :], in_=logitU_in[:, :])
            nc.sync.dma_start(out=ident_sb[:, :], in_=ident_in[:, :])
            nc.sync.dma_start(out=maskexp_sb[:, :, :], in_=maskexp_in[:, :, :])

            # warm up the collective path (first CC op pays ~30us of ring
            # init) while the build matmuls run
            ccw = dp.tile([128, 1], f32, tag="ccw")
            ccwo = dp.tile([SHARDS, 128, 1], f32, tag="ccwo")
            nc.sync.dma_start(out=ccw[:, :], in_=biasJ_sb[:, 0:1])
            nc.gpsimd.collective_compute(
                "AllGather", ALU.bypass, replica_groups=RG,
                ins=[ccw[:].opt()], outs=[ccwo[:].opt()],
            )

            # ================= build pass =================
            # jlhsT/irhs and the build bounce live in build-scoped pools;
            # their SBUF is reclaimed for the deeper pass-phase bounce ring.
            with (
                tc.tile_pool(name="bconst", bufs=1) as bcp,
                tc.tile_pool(name="bbounce", bufs=2) as bbl,
                tc.tile_pool(name="bpsum", bufs=2, space="PSUM") as bpp,
            ):
                jlhsT_sb = bcp.tile([KDIM, N], f16, tag="jlhsT")
                irhs_sb = bcp.tile([KDIM, ROWS], f16, tag="irhs")
                nc.sync.dma_start(out=jlhsT_sb[:, :], in_=jlhsT_in[:, :])
                nc.sync.dma_start(out=irhs_sb[:, :], in_=irhs_in[:, :])
                for jb in range(JB):
                    resident = jb >= NSTREAM
                    lhs = jlhsT_sb[:, jb * 128:(jb + 1) * 128]
                    bt = None
                    if not resident:
                        bt = bbl.tile([128, ROWS], f16, tag="bounce")
                    for ci, (c0, cw) in enumerate(BCHUNKS):
                        ps = bpp.tile([128, 1152], f32, tag="bps")
                        for s0, sw in BSUB:
                            nc.tensor.matmul(
                                ps[:, s0:s0 + sw], lhs,
                                irhs_sb[:, c0 + s0:c0 + s0 + sw],
                                start=True, stop=True,
                            )
                        if resident:
                            base = (jb - NSTREAM) * ROWS + c0
                            dst = S16[:, base:base + cw]
                        else:
                            dst = bt[:, c0:c0 + cw]
                        nc.scalar.activation(
                            dst, ps[:, :cw], AF.Exp,
                            bias=biasJ_sb[:, jb:jb + 1], scale=SCALE,
                            accum_out=pacc[:, jb, ci:ci + 1],
                        )
                    if not resident:
                        nc.sync.dma_start(out=S_dram[jb, :, :], in_=bt[:, :])

            # ---- deg: local partials + 4-core AllReduce ----
            pdeg = vp.tile([128, JB], f32, tag="pdeg")
            nc.vector.tensor_reduce(pdeg[:, :], pacc[:, :, :], AX.X, ALU.add)
            pd_d = dp.tile([128, JB], f32, tag="pd")
            pd_o = dp.tile([128, JB], f32, tag="pdo")
            nc.sync.dma_start(out=pd_d[:, :], in_=pdeg[:, :])
            nc.gpsimd.collective_compute(
                "AllReduce", ALU.add, replica_groups=RG,
                ins=[pd_d[:].opt()], outs=[pd_o[:].opt()],
            )
            nc.sync.dma_start(out=deg_sb[:, :], in_=pd_o[:, :])

            ln_deg = vp.tile([128, JB], f32, tag="lndeg")
            nc.scalar.activation(ln_deg[:, :], deg_sb[:, :], AF.Ln)
            nc.scalar.activation(rsqd[:, :, :], ln_deg[:, :], AF.Exp, scale=-0.5)

            # per-i coefficients from the exact deg: select this core's
            # shard columns via the host-provided one-hot mask
            masked = vp.tile([128, SHARDS, NT], f32, tag="maskd")
            nc.vector.tensor_mul(masked[:, :, :], rsqd[:, :, :],
                                 maskexp_sb[:, :, :])
            nc.vector.tensor_reduce(rsqd_l[:, :],
                                    masked[:, :, :].transpose([0, 2, 1]),
                                    AX.X, ALU.add)
            nc.vector.tensor_scalar_mul(Acoef[:, :], rsqd_l[:, :], 10.0)

            def split_hi_lo(src_f32, dst_hi, dst_lo):
                """dst_hi = f16(src); dst_lo = f16(src - dst_hi); dsts strided."""
                nc.vector.tensor_copy(dst_hi, src_f32)
                lo = vp.tile([128, JB], f32, tag="splo")
                nc.vector.tensor_sub(lo[:, :], src_f32, dst_hi)
                nc.vector.tensor_copy(dst_lo, lo[:, :])

            # pass-1 weights: [rsqd_hi, rsqd_lo, (rsqd*U)_hi, (rsqd*U)_lo]
            wq0 = vp.tile([128, JB], f32, tag="wq0")
            nc.vector.tensor_mul(wq0[:, :], rsqd[:, :, :], Ufull_sb[:, :])
            split_hi_lo(rsqd[:, :, :], qw4[:, :, 0], qw4[:, :, 1])
            split_hi_lo(wq0[:, :], qw4[:, :, 2], qw4[:, :, 3])

            # Slot schedule: main region ordered by gather-group (t//4);
            # within a group, streamed/resident slots merge proportionally so
            # the stream DMA hides under resident compute.  The last RSTAG
            # resident slots form the chunk-major stagger tail.
            def slot_schedule():
                res = list(range(NSTREAM, JB))
                stag = []
                for tg in range(5):
                    cand = [jb for jb in res
                            if tg * 4 <= (jb % NT) < min(tg * 4 + 4, NT)]
                    cand.sort(key=lambda jb: -(jb % NT))
                    stag.extend(cand[:STAG_PICK[tg]])
                assert len(stag) == RSTAG
                sset = set(stag)
                main = []
                for t0, tw in ((0, 4), (4, 4), (8, 4), (12, 4), (16, 2)):
                    grp = [jb for jb in range(JB)
                           if t0 <= (jb % NT) < t0 + tw and jb not in sset]
                    S = [jb for jb in grp if jb < NSTREAM]
                    R = [jb for jb in grp if jb >= NSTREAM]
                    ns, nr = len(S), len(R)
                    i = j = 0
                    # group 0 leads with residents: covers the first streamed
                    # DMAs at a pass boundary
                    lead = 2 if t0 == 0 else 0
                    while j < nr and j < lead:
                        main.append(R[j])
                        j += 1
                    while i < ns or j < nr:
                        if i < ns and (j >= nr or i * (nr + 1) <= j * (ns + 1)):
                            main.append(S[i])
                            i += 1
                        elif j < nr:
                            main.append(R[j])
                            j += 1
                return main, stag

            # ================= matvec passes =================
            with (
                tc.tile_pool(name="rpsum", bufs=1, space="PSUM") as rpp,
                tc.tile_pool(name="tpsum", bufs=2, space="PSUM") as tpp,
                tc.tile_pool(name="qwp", bufs=2) as qwp,
                tc.tile_pool(name="bounce", bufs=5) as bpl,
            ):
                # one PSUM tile; odd passes use rows 0:M, even rows 32:32+M
                Rps_all = rpp.tile([34, ROWS], f32, tag="rps", name="RpsAll")

                def matvec_pass(M, wsel, finish_chunk, prow):
                    """R[0:M] accumulated over all 72 j-blocks into PSUM rows
                    [prow, prow+M); transposed result lands in Tsb
                    [128, NT, :M]; finish_chunk(Tsb, nt0, ntw) runs staggered
                    per chunk and may return a deferred emitter (the gather
                    read-back + f16 weight split), which is emitted with a
                    2-chunk lag so it only waits on its own collective
                    (~6.5us completion latency) and the next pass's first
                    matmuls see their weights ready before this pass ends."""
                    Rps = Rps_all[prow:prow + M, :]
                    Tps = tpp.tile([128, NT, 4], f32, tag="tps", name="Tps")
                    Tsb = vp.tile([128, NT, 4], f32, tag="tsb", name="Tsb")
                    main, stag = slot_schedule()
                    first = True
                    for jb in main:
                        if jb < NSTREAM:
                            st = bpl.tile([128, ROWS], f16, tag="bounce")
                            nc.sync.dma_start(out=st[:, :], in_=S_dram[jb, :, :])
                            src, base0 = st, 0
                        else:
                            src, base0 = S16, (jb - NSTREAM) * ROWS
                        for c0, cw in ICHUNKS:
                            nc.tensor.matmul(
                                Rps[0:M, c0:c0 + cw], wsel(jb, M),
                                src[:, base0 + c0:base0 + c0 + cw],
                                start=first, stop=False,
                            )
                        first = False
                    deferred = []
                    for ci, (c0, cw) in enumerate(ICHUNKS):
                        for si, jb in enumerate(stag):
                            base = (jb - NSTREAM) * ROWS + c0
                            nc.tensor.matmul(
                                Rps[0:M, c0:c0 + cw], wsel(jb, M),
                                S16[:, base:base + cw],
                                start=False, stop=(si == RSTAG - 1),
                            )
                        rg = rr.tile([4, 512], f32, tag="rring")
                        nc.vector.tensor_copy(rg[0:M, 0:cw], Rps[0:M, c0:c0 + cw])
                        nt0, ntw = c0 // 128, cw // 128
                        for tt in range(ntw):
                            nc.tensor.transpose(
                                Tps[:, nt0 + tt, 0:M],
                                rg[0:M, tt * 128:(tt + 1) * 128],
                                ident_sb[0:M, 0:M],
                            )
                        nc.vector.tensor_copy(Tsb[:, nt0:nt0 + ntw, 0:M],
                                              Tps[:, nt0:nt0 + ntw, 0:M])
                        if finish_chunk is not None:
                            d = finish_chunk(Tsb, nt0, ntw)
                            if d is not None:
                                deferred.append(d)
                        if ci >= 2 and len(deferred) > ci - 2:
                            deferred[ci - 2]()
                            deferred[ci - 2] = lambda: None
                    for d in deferred:
                        d()
                    return Tsb

                def chunk_z(Tsb, nt0, ntw, qt, m0):
                    """z = Ccoef + Acoef * (T[m0]+T[m0+1]);
                    q slice <- 1/(1+exp(-z)) (keeps ACT on the Exp set)."""
                    Rr = vp.tile([128, NT], f32, tag="Rrc")
                    nc.vector.tensor_add(Rr[:, 0:ntw], Tsb[:, nt0:nt0 + ntw, m0],
                                         Tsb[:, nt0:nt0 + ntw, m0 + 1])
                    t2 = vp.tile([128, NT], f32, tag="t2c")
                    nc.vector.tensor_mul(t2[:, 0:ntw], Acoef[:, nt0:nt0 + ntw],
                                         Rr[:, 0:ntw])
                    z = vp.tile([128, NT], f32, tag="zc")
                    nc.vector.tensor_add(z[:, 0:ntw], Ccoef[:, nt0:nt0 + ntw],
                                         t2[:, 0:ntw])
                    ez = vp.tile([128, NT], f32, tag="ezc")
                    nc.scalar.activation(ez[:, 0:ntw], z[:, 0:ntw],
                                         AF.Exp, scale=-1.0)
                    e1 = vp.tile([128, NT], f32, tag="e1c")
                    nc.vector.tensor_scalar_add(e1[:, 0:ntw], ez[:, 0:ntw], 1.0)
                    nc.vector.reciprocal(qt[:, nt0:nt0 + ntw], e1[:, 0:ntw])

                def subgather_start(qt, nt0, ntw, qwg):
                    """Issue the collective for q t-columns [nt0, nt0+ntw).
                    Returns an emitter for the qc read-back DMA + f16 weight
                    split into qwg (the next pass's weights)."""
                    wsp = gp.tile([128, ntw], f32, tag="wspc")
                    qg = gp.tile([SHARDS, 128, ntw], f32, tag="qgc")
                    nc.scalar.dma_start(out=wsp[:, :], in_=qt[:, nt0:nt0 + ntw])
                    nc.gpsimd.collective_compute(
                        "AllGather", ALU.bypass, replica_groups=RG,
                        ins=[wsp[:].opt()], outs=[qg[:].opt()],
                    )

                    def emit_tail():
                        # qc rides the GpSimd queue: the Tile scheduler
                        # re-orders per-queue streams, and on Scalar/Sync this
                        # DMA's wait on the collective would stall z-exps or
                        # the bounce stream behind it.  The GpSimd queue only
                        # carries collective launches, which are already
                        # serialized by the CC core.
                        qc = vp.tile([128, SHARDS, NT4], f32, tag=f"qc{nt0 // 4}",
                                     name="qc")
                        nc.gpsimd.dma_start(out=qc[:, :, 0:ntw],
                                            in_=qg[:, :, :].transpose([1, 0, 2]))
                        subgather_finish(qc, nt0, ntw, qwg)
                    return emit_tail

                def subgather_finish(qc, nt0, ntw, qwg):
                    """DVE half of the gather: w = rsqd*q, split hi/lo f16."""
                    wqc = vp.tile([128, SHARDS, NT4], f32, tag="wqc")
                    nc.vector.tensor_mul(wqc[:, :, 0:ntw],
                                         rsqd[:, :, nt0:nt0 + ntw],
                                         qc[:, :, 0:ntw])
                    nc.vector.tensor_copy(qwg[:, :, 0:ntw, 0],
                                          wqc[:, :, 0:ntw])
                    spl = vp.tile([128, SHARDS, NT4], f32, tag="splc")
                    nc.vector.tensor_sub(spl[:, :, 0:ntw], wqc[:, :, 0:ntw],
                                         qwg[:, :, 0:ntw, 0])
                    nc.vector.tensor_copy(qwg[:, :, 0:ntw, 1],
                                          spl[:, :, 0:ntw])

                def alloc_qw():
                    """One weight tile per gather-group (t//4)."""
                    tiles = []
                    for gi in range(5):
                        qwg = qwp.tile([128, SHARDS, NT4, 2], f16,
                                       tag=f"qw{gi}", name=f"qwg{gi}")
                        tiles.append(qwg)
                    return tiles

                def wsel_of(qws):
                    def wsel(jb, M):
                        g, t = divmod(jb, NT)
                        return qws[t // 4][:, g, t % 4, 0:M]
                    return wsel

                # ---- pass 1: tvec + iteration 1 (M=4) ----
                wsel4 = lambda jb, M: qw4[:, jb, 0:M]
                q1 = vp.tile([128, NT], f32, tag="qpass", name="q1")
                qw_cur = alloc_qw()

                def fin1(Tsb, nt0, ntw, qt=q1, qws=qw_cur):
                    tv = vp.tile([128, NT], f32, tag="tvc")
                    nc.vector.tensor_add(tv[:, 0:ntw], Tsb[:, nt0:nt0 + ntw, 0],
                                         Tsb[:, nt0:nt0 + ntw, 1])
                    tmpc = vp.tile([128, NT], f32, tag="tmpc")
                    nc.vector.tensor_mul(tmpc[:, 0:ntw], rsqd_l[:, nt0:nt0 + ntw],
                                         tv[:, 0:ntw])
                    tm2c = vp.tile([128, NT], f32, tag="tm2c")
                    nc.vector.tensor_scalar_mul(tm2c[:, 0:ntw], tmpc[:, 0:ntw],
                                                -5.0)
                    nc.vector.tensor_add(Ccoef[:, nt0:nt0 + ntw],
                                         logitU_sb[:, nt0:nt0 + ntw],
                                         tm2c[:, 0:ntw])
                    chunk_z(Tsb, nt0, ntw, qt, 2)
                    return subgather_start(qt, nt0, ntw, qws[nt0 // 4])

                matvec_pass(4, wsel4, fin1, prow=0)

                # ---- passes 2..11 ----
                for it in range(1, REFINE_ITERS):
                    last = (it == REFINE_ITERS - 1)
                    wsel2 = wsel_of(qw_cur)
                    qt = vp.tile([128, NT], f32, tag="qpass", name="qt")
                    if not last:
                        qw_cur = alloc_qw()

                    def fin(Tsb, nt0, ntw, qt=qt, last=last,
                            qws=(None if last else qw_cur)):
                        chunk_z(Tsb, nt0, ntw, qt, 0)
                        if last:
                            nc.sync.dma_start(out=q_out_d[:, nt0:nt0 + ntw],
                                              in_=qt[:, nt0:nt0 + ntw])
                            return None
                        return subgather_start(qt, nt0, ntw, qws[nt0 // 4])

                    matvec_pass(2, wsel2, fin, prow=(32 if it % 2 else 0))

    nc.compile()
    return nc


_NC_CACHE = None


def kernel(imgs, masks):
    global _NC_CACHE
    from concourse.bass_utils import run_bass_kernel_spmd

    in_maps = make_in_maps(imgs, masks)
    if _NC_CACHE is None:
        _NC_CACHE = build_program()
    res = run_bass_kernel_spmd(_NC_CACHE, in_maps, list(range(N_CORES)))
    return assemble(res.results)


# revision 20
# speedup vs baseline: 1.0824x; 1.0117x over previous
"""Trainium2 Bass kernel v3 for CRFHead (dense-Gaussian mean-field CRF).

v2 materialized E = exp(-0.5 d2) once per core in f16 (34/72 j-blocks
SBUF-resident, 38 streamed via DRAM) and ran 11 TensorE matvec passes with
PSUM accumulating across 72 j-block matmuls.  v3 keeps that skeleton and
removes the scheduling stalls found in the v2 trace:

- ~11us inter-pass bubbles: the next pass's first matmul chained through
  [group-4 collective -> qc DMA -> counting-sem -> group-0 weight split].
  v3 emits the qc DMAs with a 2-chunk lag on the Scalar queue and the
  f16 weight splits at the END of the producing pass, so pass k+1's
  weights are ready before pass k's last matmul retires.
- PSUM WAW serialization at pass boundaries: Rps is one [34, ROWS] tile;
  odd passes accumulate into rows 0:M, even passes into rows 32:32+M, so
  the first matmuls of pass k+1 don't wait for pass k's PSUM readers.
- ACT table ping-pong (Ln/Exp per chunk in pass 1's tail, 1.28us per
  reload on the Scalar queue): the per-i 1/sqrt(deg) coefficients now
  come from the exact AllReduced deg via a host-provided shard mask
  (masked reduce over the SHARDS axis) instead of riding pass 1 as a
  ones-weight column; pass 1 shrinks to M=4 and uses only Exp.

Sharding: 8 cores = 2 images x 4-way split of the 9216 output pixels.
"""

import numpy as np

B, C, H, W = 2, 3, 96, 96
N = H * W                 # 9216 pixels
N_CORES = 8
SHARDS = 4                # cores per image
ROWS = N // SHARDS        # 2304 local output rows per core
NT = ROWS // 128          # 18 local 128-row tiles
JB = N // 128             # 72 global j-blocks
NRES = 36                 # j-blocks resident in SBUF (slots NSTREAM..71)
NSTREAM = JB - NRES       # j-blocks streamed from DRAM (slots 0..NSTREAM-1)
KDIM = 12
REFINE_ITERS = 10
RG = [[0, 1, 2, 3], [4, 5, 6, 7]]
ICHUNKS = [(0, 512), (512, 512), (1024, 512), (1536, 512), (2048, 256)]
BCHUNKS = [(0, 1152), (1152, 1152)]
BSUB = [(0, 512), (512, 512), (1024, 128)]
RSTAG = 12                # resident slots reserved for the chunk-major tail
STAG_PICK = (0, 3, 4, 3, 2)
NT4 = 4                   # max t-tiles per i-chunk

F16 = np.float16


def _f16(x):
    return np.asarray(x, dtype=F16).astype(np.float32)


def _split3_f16(w):
    w = np.asarray(w, np.float32)
    w1 = np.asarray(w, F16)
    d1 = w - w1.astype(np.float32)
    w2 = np.asarray(d1, F16)
    w3 = np.asarray(d1 - w2.astype(np.float32), F16)
    return w1, w2, w3


def _host_prep(imgs, masks):
    """Mirror the reference's quantization exactly in numpy fp32."""
    imgs = np.asarray(imgs, np.float32)
    masks = np.asarray(masks, np.float32)
    MEAN = np.array([0.485, 0.456, 0.406], np.float32)[None, :, None, None]
    STD = np.array([0.229, 0.224, 0.225], np.float32)[None, :, None, None]
    x = (imgs * STD + MEAN).transpose(0, 2, 3, 1) * np.float32(255.0)
    x = np.floor(np.clip(x, 0.0, 255.0))
    m = np.floor(np.clip(masks * np.float32(255.0) / np.float32(0.7), 0.0, 255.0))
    return x, m


def _image_data(img_q, mask_q):
    """Per-image full-N host arrays (global row order = row-major pixels)."""
    U = mask_q / (mask_q.max() + 1e-8)
    U = np.clip(U, 1e-6, 1.0 - 1e-6).reshape(N).astype(np.float32)
    logitU = np.log(U / (np.float32(1.0) - U)).astype(np.float32)

    ys, xs = np.meshgrid(np.arange(H, dtype=np.float32),
                         np.arange(W, dtype=np.float32), indexing="ij")
    xv = xs.reshape(N)
    yv = ys.reshape(N)
    c = img_q.reshape(N, 3).astype(np.float32)

    ax = (xv / np.float32(12.0)).astype(np.float32)
    ay = (yv / np.float32(12.0)).astype(np.float32)
    axh = _f16(ax); axl = _f16(ax - axh)
    ayh = _f16(ay); ayl = _f16(ay - ayh)
    r, g, b = _f16(c[:, 0]), _f16(c[:, 1]), _f16(c[:, 2])
    twos = np.full(N, 2.0, np.float32)

    sqxy = xv * xv + yv * yv
    sqrgb = (c * c).sum(axis=1)
    bias = (-sqrgb / np.float32(50.0) - sqxy / np.float32(7200.0)).astype(np.float32)
    whalf = (np.float32(12.5) * bias).astype(np.float32)   # w_i/2; ones-row = 2

    # stationary j-side rows x moving i-side rows -> PSUM holds
    # 25*(f_i . f_j) + w_i ; ACT applies scale 1/25 and per-partition bias_j
    j_rows = np.stack([r, g, b, axh, axh, axl, ayh, ayh, ayl,
                       twos, twos, twos]).astype(F16)           # [12, N]
    i_rows = np.stack([r, g, b, axh, axl, axh, ayh, ayl, ayh,
                       *_split3_f16(whalf)]).astype(F16)        # [12, N]
    return dict(U=U, logitU=logitU, bias=bias,
                j_rows=j_rows, i_rows=i_rows)


def _pb_index():
    """Global row index for [p, jb] layouts: j = (jb//NT)*ROWS + (jb%NT)*128 + p."""
    p = np.arange(128)[:, None]
    jb = np.arange(JB)[None, :]
    return (jb // NT) * ROWS + (jb % NT) * 128 + p      # [128, 72]


def _core_inputs(data, g):
    gidx = _pb_index()
    jlhsT = data["j_rows"][:, gidx.T.reshape(-1)]                  # [12, 72*128]
    isl = slice(g * ROWS, (g + 1) * ROWS)
    irhs = data["i_rows"][:, isl]                                  # [12, 2304]
    biasJ = np.ascontiguousarray(data["bias"][gidx], np.float32)   # [128, 72]
    Ufull = np.ascontiguousarray(data["U"][gidx], np.float32)      # [128, 72]
    lidx = gidx[:, g * NT:(g + 1) * NT]
    logitU = np.ascontiguousarray(data["logitU"][lidx], np.float32)  # [128, 18]
    ident = np.eye(8, dtype=np.float32)
    # one-hot shard mask, [128, SHARDS, NT] flattened as [128, JB]
    maskexp = np.zeros((128, SHARDS, NT), np.float32)
    maskexp[:, g, :] = 1.0
    return {
        "jlhsT": np.ascontiguousarray(jlhsT),
        "irhs": np.ascontiguousarray(irhs),
        "biasJ": biasJ,
        "Ufull": Ufull,
        "logitUl": logitU,
        "ident": ident,
        "maskexp": np.ascontiguousarray(maskexp),
    }


def make_in_maps(imgs, masks):
    x, m = _host_prep(imgs, masks)
    per_image = [_image_data(x[b], m[b]) for b in range(B)]
    in_maps = []
    for k in range(N_CORES):
        b, g = divmod(k, SHARDS)
        in_maps.append(_core_inputs(per_image[b], g))
    return in_maps


def assemble(results):
    out = np.empty((B, N), np.float32)
    p = np.arange(128)[:, None]
    t = np.arange(NT)[None, :]
    lidx = (t * 128 + p).reshape(-1)
    for k in range(N_CORES):
        b, g = divmod(k, SHARDS)
        flat = np.empty(ROWS, np.float32)
        flat[lidx] = np.asarray(results[k]["q_out"], np.float32).reshape(-1)
        out[b, g * ROWS:(g + 1) * ROWS] = flat
    return out.reshape(B, H, W)


def build_program():
    import concourse.bacc as bacc
    import concourse.mybir as mybir
    from concourse.tile import TileContext

    f32 = mybir.dt.float32
    f16 = mybir.dt.float16
    AF = mybir.ActivationFunctionType
    AX = mybir.AxisListType
    ALU = mybir.AluOpType

    nc = bacc.Bacc(num_devices=N_CORES)

    jlhsT_in = nc.dram_tensor("jlhsT", [KDIM, N], f16, kind="ExternalInput")
    irhs_in = nc.dram_tensor("irhs", [KDIM, ROWS], f16, kind="ExternalInput")
    biasJ_in = nc.dram_tensor("biasJ", [128, JB], f32, kind="ExternalInput")
    Ufull_in = nc.dram_tensor("Ufull", [128, JB], f32, kind="ExternalInput")
    logitU_in = nc.dram_tensor("logitUl", [128, NT], f32, kind="ExternalInput")
    ident_in = nc.dram_tensor("ident", [8, 8], f32, kind="ExternalInput")
    maskexp_in = nc.dram_tensor("maskexp", [128, SHARDS, NT], f32,
                                kind="ExternalInput")
    q_out_d = nc.dram_tensor("q_out", [128, NT], f32, kind="ExternalOutput")

    SCALE = float(np.float32(1.0) / np.float32(25.0))

    with TileContext(nc) as tc:
        with (
            tc.tile_pool(name="const", bufs=1) as cpool,
            tc.tile_pool(name="vec", bufs=2) as vp,
            tc.tile_pool(name="ring", bufs=2) as rr,
            tc.tile_pool(name="sdram", bufs=1, space="DRAM") as dpc,
            tc.tile_pool(name="dramit", bufs=2, space="DRAM") as dp,
            tc.tile_pool(name="gdram", bufs=5, space="DRAM") as gp,
        ):
            # ---- persistent SBUF ----
            biasJ_sb = cpool.tile([128, JB], f32, tag="biasJ")
            Ufull_sb = cpool.tile([128, JB], f32, tag="Ufull")
            logitU_sb = cpool.tile([128, NT], f32, tag="logitU")
            ident_sb = cpool.tile([8, 8], f32, tag="ident")
            maskexp_sb = cpool.tile([128, SHARDS, NT], f32, tag="maskexp")
            S16 = cpool.tile([128, NRES * ROWS], f16, tag="S16")
            pacc = cpool.tile([128, JB, 2], f32, tag="pacc")
            deg_sb = cpool.tile([128, JB], f32, tag="deg")
            rsqd = cpool.tile([128, SHARDS, NT], f32, tag="rsqd")
            qw4 = cpool.tile([128, JB, 4], f16, tag="qw4")
            rsqd_l = cpool.tile([128, NT], f32, tag="rsqdl")
            Acoef = cpool.tile([128, NT], f32, tag="Acoef")
            Ccoef = cpool.tile([128, NT], f32, tag="Ccoef")

            S_dram = dpc.tile([NSTREAM, 128, ROWS], f16, tag="Sdram")

            nc.sync.dma_start(out=biasJ_sb[:, :], in_=biasJ_in[:, :])
            nc.sync.dma_start(out=Ufull_sb[:, :], in_=Ufull_in[:, :])
            nc.sync.dma_start(out=logitU_sb[:, :], in_=logitU_in[:, :])
            nc.sync.dma_start(out=ident_sb[:, :], in_=ident_in[:, :])
            nc.sync.dma_start(out=maskexp_sb[:, :, :], in_=maskexp_in[:, :, :])

            # warm up the collective path (first CC op pays ~30us of ring
            # init) while the build matmuls run
            ccw = dp.tile([128, 1], f32, tag="ccw")
            ccwo = dp.tile([SHARDS, 128, 1], f32, tag="ccwo")
            nc.sync.dma_start(out=ccw[:, :], in_=biasJ_sb[:, 0:1])
            nc.gpsimd.collective_compute(
                "AllGather", ALU.bypass, replica_groups=RG,
                ins=[ccw[:].opt()], outs=[ccwo[:].opt()],
            )

            # ================= build pass =================
            # jlhsT/irhs and the build bounce live in build-scoped pools;
            # their SBUF is reclaimed for the deeper pass-phase bounce ring.
            with (
                tc.tile_pool(name="bconst", bufs=1) as bcp,
                tc.tile_pool(name="bbounce", bufs=2) as bbl,
                tc.tile_pool(name="bpsum", bufs=2, space="PSUM") as bpp,
            ):
                jlhsT_sb = bcp.tile([KDIM, N], f16, tag="jlhsT")
                irhs_sb = bcp.tile([KDIM, ROWS], f16, tag="irhs")
                nc.sync.dma_start(out=jlhsT_sb[:, :], in_=jlhsT_in[:, :])
                nc.sync.dma_start(out=irhs_sb[:, :], in_=irhs_in[:, :])
                for jb in range(JB):
                    resident = jb >= NSTREAM
                    lhs = jlhsT_sb[:, jb * 128:(jb + 1) * 128]
                    bt = None
                    if not resident:
                        bt = bbl.tile([128, ROWS], f16, tag="bounce")
                    for ci, (c0, cw) in enumerate(BCHUNKS):
                        ps = bpp.tile([128, 1152], f32, tag="bps")
                        for s0, sw in BSUB:
                            nc.tensor.matmul(
                                ps[:, s0:s0 + sw], lhs,
                                irhs_sb[:, c0 + s0:c0 + s0 + sw],
                                start=True, stop=True,
                            )
                        if resident:
                            base = (jb - NSTREAM) * ROWS + c0
                            dst = S16[:, base:base + cw]
                        else:
                            dst = bt[:, c0:c0 + cw]
                        nc.scalar.activation(
                            dst, ps[:, :cw], AF.Exp,
                            bias=biasJ_sb[:, jb:jb + 1], scale=SCALE,
                            accum_out=pacc[:, jb, ci:ci + 1],
                        )
                        if not resident:
                            # per-chunk write frees the bounce slot ~2us
                            # earlier than a whole-block write
                            nc.sync.dma_start(out=S_dram[jb, :, c0:c0 + cw],
                                              in_=bt[:, c0:c0 + cw])

            # ---- deg: local partials + 4-core AllReduce ----
            pdeg = vp.tile([128, JB], f32, tag="pdeg")
            nc.vector.tensor_reduce(pdeg[:, :], pacc[:, :, :], AX.X, ALU.add)
            pd_d = dp.tile([128, JB], f32, tag="pd")
            pd_o = dp.tile([128, JB], f32, tag="pdo")
            nc.sync.dma_start(out=pd_d[:, :], in_=pdeg[:, :])
            nc.gpsimd.collective_compute(
                "AllReduce", ALU.add, replica_groups=RG,
                ins=[pd_d[:].opt()], outs=[pd_o[:].opt()],
            )
            nc.sync.dma_start(out=deg_sb[:, :], in_=pd_o[:, :])

            ln_deg = vp.tile([128, JB], f32, tag="lndeg")
            nc.scalar.activation(ln_deg[:, :], deg_sb[:, :], AF.Ln)
            nc.scalar.activation(rsqd[:, :, :], ln_deg[:, :], AF.Exp, scale=-0.5)

            # per-i coefficients from the exact deg: select this core's
            # shard columns via the host-provided one-hot mask
            masked = vp.tile([128, SHARDS, NT], f32, tag="maskd")
            nc.vector.tensor_mul(masked[:, :, :], rsqd[:, :, :],
                                 maskexp_sb[:, :, :])
            nc.vector.tensor_reduce(rsqd_l[:, :],
                                    masked[:, :, :].transpose([0, 2, 1]),
                                    AX.X, ALU.add)
            nc.vector.tensor_scalar_mul(Acoef[:, :], rsqd_l[:, :], 10.0)

            def split_hi_lo(src_f32, dst_hi, dst_lo):
                """dst_hi = f16(src); dst_lo = f16(src - dst_hi); dsts strided."""
                nc.vector.tensor_copy(dst_hi, src_f32)
                lo = vp.tile([128, JB], f32, tag="splo")
                nc.vector.tensor_sub(lo[:, :], src_f32, dst_hi)
                nc.vector.tensor_copy(dst_lo, lo[:, :])

            # pass-1 weights: [rsqd_hi, rsqd_lo, (rsqd*U)_hi, (rsqd*U)_lo]
            wq0 = vp.tile([128, JB], f32, tag="wq0")
            nc.vector.tensor_mul(wq0[:, :], rsqd[:, :, :], Ufull_sb[:, :])
            split_hi_lo(rsqd[:, :, :], qw4[:, :, 0], qw4[:, :, 1])
            split_hi_lo(wq0[:, :], qw4[:, :, 2], qw4[:, :, 3])

            # Slot schedule: main region ordered by gather-group (t//4);
            # within a group, streamed/resident slots merge proportionally so
            # the stream DMA hides under resident compute.  The last RSTAG
            # resident slots form the chunk-major stagger tail.
            def slot_schedule():
                res = list(range(NSTREAM, JB))
                stag = []
                for tg in range(5):
                    cand = [jb for jb in res
                            if tg * 4 <= (jb % NT) < min(tg * 4 + 4, NT)]
                    cand.sort(key=lambda jb: -(jb % NT))
                    stag.extend(cand[:STAG_PICK[tg]])
                assert len(stag) == RSTAG
                sset = set(stag)
                main = []
                for t0, tw in ((0, 4), (4, 4), (8, 4), (12, 4), (16, 2)):
                    grp = [jb for jb in range(JB)
                           if t0 <= (jb % NT) < t0 + tw and jb not in sset]
                    S = [jb for jb in grp if jb < NSTREAM]
                    R = [jb for jb in grp if jb >= NSTREAM]
                    ns, nr = len(S), len(R)
                    i = j = 0
                    # group 0 leads with residents: covers the first streamed
                    # DMAs at a pass boundary
                    lead = 2 if t0 == 0 else 0
                    while j < nr and j < lead:
                        main.append(R[j])
                        j += 1
                    while i < ns or j < nr:
                        if i < ns and (j >= nr or i * (nr + 1) <= j * (ns + 1)):
                            main.append(S[i])
                            i += 1
                        elif j < nr:
                            main.append(R[j])
                            j += 1
                return main, stag

            # ================= matvec passes =================
            with (
                tc.tile_pool(name="rpsum", bufs=1, space="PSUM") as rpp,
                tc.tile_pool(name="tpsum", bufs=2, space="PSUM") as tpp,
                tc.tile_pool(name="qwp", bufs=2) as qwp,
                tc.tile_pool(name="bounce", bufs=5) as bpl,
            ):
                # one PSUM tile; odd passes use rows 0:M, even rows 32:32+M
                Rps_all = rpp.tile([34, ROWS], f32, tag="rps", name="RpsAll")

                def matvec_pass(M, wsel, finish_chunk, prow):
                    """R[0:M] accumulated over all 72 j-blocks into PSUM rows
                    [prow, prow+M); transposed result lands in Tsb
                    [128, NT, :M]; finish_chunk(Tsb, nt0, ntw) runs staggered
                    per chunk and may return a deferred emitter (the gather
                    read-back + f16 weight split), which is emitted with a
                    2-chunk lag so it only waits on its own collective
                    (~6.5us completion latency) and the next pass's first
                    matmuls see their weights ready before this pass ends."""
                    Rps = Rps_all[prow:prow + M, :]
                    Tps = tpp.tile([128, NT, 4], f32, tag="tps", name="Tps")
                    Tsb = vp.tile([128, NT, 4], f32, tag="tsb", name="Tsb")
                    main, stag = slot_schedule()
                    first = True
                    for jb in main:
                        if jb < NSTREAM:
                            st = bpl.tile([128, ROWS], f16, tag="bounce")
                            nc.sync.dma_start(out=st[:, :], in_=S_dram[jb, :, :])
                            src, base0 = st, 0
                        else:
                            src, base0 = S16, (jb - NSTREAM) * ROWS
                        for c0, cw in ICHUNKS:
                            nc.tensor.matmul(
                                Rps[0:M, c0:c0 + cw], wsel(jb, M),
                                src[:, base0 + c0:base0 + c0 + cw],
                                start=first, stop=False,
                            )
                        first = False
                    deferred = []
                    for ci, (c0, cw) in enumerate(ICHUNKS):
                        for si, jb in enumerate(stag):
                            base = (jb - NSTREAM) * ROWS + c0
                            nc.tensor.matmul(
                                Rps[0:M, c0:c0 + cw], wsel(jb, M),
                                S16[:, base:base + cw],
                                start=False, stop=(si == RSTAG - 1),
                            )
                        rg = rr.tile([4, 512], f32, tag="rring")
                        nc.vector.tensor_copy(rg[0:M, 0:cw], Rps[0:M, c0:c0 + cw])
                        nt0, ntw = c0 // 128, cw // 128
                        for tt in range(ntw):
                            nc.tensor.transpose(
                                Tps[:, nt0 + tt, 0:M],
                                rg[0:M, tt * 128:(tt + 1) * 128],
                                ident_sb[0:M, 0:M],
                            )
                        nc.vector.tensor_copy(Tsb[:, nt0:nt0 + ntw, 0:M],
                                              Tps[:, nt0:nt0 + ntw, 0:M])
                        if finish_chunk is not None:
                            d = finish_chunk(Tsb, nt0, ntw)
                            if d is not None:
                                deferred.append(d)
                        if ci >= 2 and len(deferred) > ci - 2:
                            deferred[ci - 2]()
                            deferred[ci - 2] = lambda: None
                    for d in deferred:
                        d()
                    return Tsb

                def chunk_z(Tsb, nt0, ntw, qt, m0):
                    """z = Ccoef + Acoef * (T[m0]+T[m0+1]);
                    q slice <- 1/(1+exp(-z)) (keeps ACT on the Exp set)."""
                    Rr = vp.tile([128, NT], f32, tag="Rrc")
                    nc.vector.tensor_add(Rr[:, 0:ntw], Tsb[:, nt0:nt0 + ntw, m0],
                                         Tsb[:, nt0:nt0 + ntw, m0 + 1])
                    t2 = vp.tile([128, NT], f32, tag="t2c")
                    nc.vector.tensor_mul(t2[:, 0:ntw], Acoef[:, nt0:nt0 + ntw],
                                         Rr[:, 0:ntw])
                    z = vp.tile([128, NT], f32, tag="zc")
                    nc.vector.tensor_add(z[:, 0:ntw], Ccoef[:, nt0:nt0 + ntw],
                                         t2[:, 0:ntw])
                    ez = vp.tile([128, NT], f32, tag="ezc")
                    nc.scalar.activation(ez[:, 0:ntw], z[:, 0:ntw],
                                         AF.Exp, scale=-1.0)
                    e1 = vp.tile([128, NT], f32, tag="e1c")
                    nc.vector.tensor_scalar_add(e1[:, 0:ntw], ez[:, 0:ntw], 1.0)
                    nc.vector.reciprocal(qt[:, nt0:nt0 + ntw], e1[:, 0:ntw])

                def subgather_start(qt, nt0, ntw, qwg):
                    """Issue the collective for q t-columns [nt0, nt0+ntw).
                    Returns an emitter for the qc read-back DMA + f16 weight
                    split into qwg (the next pass's weights)."""
                    wsp = gp.tile([128, ntw], f32, tag="wspc")
                    qg = gp.tile([SHARDS, 128, ntw], f32, tag="qgc")
                    nc.scalar.dma_start(out=wsp[:, :], in_=qt[:, nt0:nt0 + ntw])
                    nc.gpsimd.collective_compute(
                        "AllGather", ALU.bypass, replica_groups=RG,
                        ins=[wsp[:].opt()], outs=[qg[:].opt()],
                    )

                    def emit_tail():
                        # qc rides the GpSimd queue: the Tile scheduler
                        # re-orders per-queue streams, and on Scalar/Sync this
                        # DMA's wait on the collective would stall z-exps or
                        # the bounce stream behind it.  The GpSimd queue only
                        # carries collective launches, which are already
                        # serialized by the CC core.
                        qc = vp.tile([128, SHARDS, NT4], f32, tag=f"qc{nt0 // 4}",
                                     name="qc")
                        nc.gpsimd.dma_start(out=qc[:, :, 0:ntw],
                                            in_=qg[:, :, :].transpose([1, 0, 2]))
                        subgather_finish(qc, nt0, ntw, qwg)
                    return emit_tail

                def subgather_finish(qc, nt0, ntw, qwg):
                    """DVE half of the gather: w = rsqd*q, split hi/lo f16."""
                    wqc = vp.tile([128, SHARDS, NT4], f32, tag="wqc")
                    nc.vector.tensor_mul(wqc[:, :, 0:ntw],
                                         rsqd[:, :, nt0:nt0 + ntw],
                                         qc[:, :, 0:ntw])
                    nc.vector.tensor_copy(qwg[:, :, 0:ntw, 0],
                                          wqc[:, :, 0:ntw])
                    spl = vp.tile([128, SHARDS, NT4], f32, tag="splc")
                    nc.vector.tensor_sub(spl[:, :, 0:ntw], wqc[:, :, 0:ntw],
                                         qwg[:, :, 0:ntw, 0])
                    nc.vector.tensor_copy(qwg[:, :, 0:ntw, 1],
                                          spl[:, :, 0:ntw])

                def alloc_qw():
                    """One weight tile per gather-group (t//4)."""
                    tiles = []
                    for gi in range(5):
                        qwg = qwp.tile([128, SHARDS, NT4, 2], f16,
                                       tag=f"qw{gi}", name=f"qwg{gi}")
                        tiles.append(qwg)
                    return tiles

                def wsel_of(qws):
                    def wsel(jb, M):
                        g, t = divmod(jb, NT)
                        return qws[t // 4][:, g, t % 4, 0:M]
                    return wsel

                # ---- pass 1: tvec + iteration 1 (M=4) ----
                wsel4 = lambda jb, M: qw4[:, jb, 0:M]
                q1 = vp.tile([128, NT], f32, tag="qpass", name="q1")
                qw_cur = alloc_qw()

                def fin1(Tsb, nt0, ntw, qt=q1, qws=qw_cur):
                    tv = vp.tile([128, NT], f32, tag="tvc")
                    nc.vector.tensor_add(tv[:, 0:ntw], Tsb[:, nt0:nt0 + ntw, 0],
                                         Tsb[:, nt0:nt0 + ntw, 1])
                    tmpc = vp.tile([128, NT], f32, tag="tmpc")
                    nc.vector.tensor_mul(tmpc[:, 0:ntw], rsqd_l[:, nt0:nt0 + ntw],
                                         tv[:, 0:ntw])
                    tm2c = vp.tile([128, NT], f32, tag="tm2c")
                    nc.vector.tensor_scalar_mul(tm2c[:, 0:ntw], tmpc[:, 0:ntw],
                                                -5.0)
                    nc.vector.tensor_add(Ccoef[:, nt0:nt0 + ntw],
                                         logitU_sb[:, nt0:nt0 + ntw],
                                         tm2c[:, 0:ntw])
                    chunk_z(Tsb, nt0, ntw, qt, 2)
                    return subgather_start(qt, nt0, ntw, qws[nt0 // 4])

                matvec_pass(4, wsel4, fin1, prow=0)

                # ---- passes 2..11 ----
                for it in range(1, REFINE_ITERS):
                    last = (it == REFINE_ITERS - 1)
                    wsel2 = wsel_of(qw_cur)
                    qt = vp.tile([128, NT], f32, tag="qpass", name="qt")
                    if not last:
                        qw_cur = alloc_qw()

                    def fin(Tsb, nt0, ntw, qt=qt, last=last,
                            qws=(None if last else qw_cur)):
                        chunk_z(Tsb, nt0, ntw, qt, 0)
                        if last:
                            nc.sync.dma_start(out=q_out_d[:, nt0:nt0 + ntw],
                                              in_=qt[:, nt0:nt0 + ntw])
                            return None
                        return subgather_start(qt, nt0, ntw, qws[nt0 // 4])

                    matvec_pass(2, wsel2, fin, prow=(32 if it % 2 else 0))

    nc.compile()
    return nc


_NC_CACHE = None


def kernel(imgs, masks):
    global _NC_CACHE
    from concourse.bass_utils import run_bass_kernel_spmd

    in_maps = make_in_maps(imgs, masks)
    if _NC_CACHE is None:
        _NC_CACHE = build_program()
    res = run_bass_kernel_spmd(_NC_CACHE, in_maps, list(range(N_CORES)))
    return assemble(res.results)
